# revision 37
# baseline (speedup 1.0000x reference)
"""MockDeepSeekAttention (MLA) fused kernel for 8 TRN2 NeuronCores.

Sharding: tensor-parallel over heads (4 heads/core) for q/kv_b/attention/
o_proj; the shared kv_a latent projection + LN + k-rope is column-sharded
across the 8 cores and all-gathered on-chip (it would otherwise be
replicated work). The host sums the 8 partial o_proj outputs.

Kernel structure (per core):
  L: latent projection for this core's 256-token column slice + LN +
     k-RoPE, packed and AllGather'd via DRAM bounce buffers. The gather
     completes in the background of phase B.
  B: q projections (nope + rope) for this core's 4 heads, N=512 chains,
     fully SBUF-resident.
  C: kv_b projections (k_nope per head, V blocks) from the gathered latent.
  D: causal attention, scores^T layout; softmax sum via ones-matmul;
     mask via sub-sliced exp + memset + [128,128] triangle multiply.
  E: partial o_proj, h-middle loop order so each stationary tile serves
     4 matmuls; bf16 partial output (host sums in f32).
"""

import sys

if "/opt/trn_rl_repo" not in sys.path:
    sys.path.insert(0, "/opt/trn_rl_repo")

import numpy as np
import ml_dtypes

BF16 = ml_dtypes.bfloat16

# Model dims (hardcoded per problem spec)
H = 32
D_NOPE = 128
D_ROPE = 64
D_V = 128
Q_HEAD = D_NOPE + D_ROPE  # 192
KV_LORA = 512
HIDDEN = 4096
S = 2048
NCORES = 8
HC = H // NCORES  # 4 heads per core
SCALE = 1.0 / Q_HEAD**0.5
LN_EPS = 1e-5

KT = HIDDEN // 128   # 32 k-tiles
NSL = 512            # column tile / attention sq tile
NN = S // NSL        # 4
KP = 8               # k-tiles per DMA piece
NPC = KT // KP       # 4 pieces per column tile
LSL = S // NCORES    # 256: per-core latent column slice

QW_COLS = HC * 128 + HC * 64  # 768
KVA_COLS = 640                # 512 latent + 128 duplicated rope

_CACHE = {}


def _build_program():
    import concourse.bass as bass
    import concourse.tile as tile
    from concourse import mybir, bacc
    from contextlib import ExitStack

    f32 = mybir.dt.float32
    bf16 = mybir.dt.bfloat16
    Exp = mybir.ActivationFunctionType.Exp
    Sqrt = mybir.ActivationFunctionType.Sqrt
    Copy = mybir.ActivationFunctionType.Copy

    nc = bacc.Bacc()
    # DRAM params, already in SBUF (partition-major) layout from the host.
    hN = nc.declare_dram_parameter("hN", [NN, NPC, 128, KP * NSL], bf16, isOutput=False)
    hL = nc.declare_dram_parameter("hL", [2, 128, 16 * LSL], bf16, isOutput=False)
    qwT = nc.declare_dram_parameter("qwT", [128, KT * QW_COLS], bf16, isOutput=False)
    kvawT = nc.declare_dram_parameter("kvawT", [128, KT * KVA_COLS], bf16, isOutput=False)
    kbw = nc.declare_dram_parameter("kbw", [128, 4 * 1024], bf16, isOutput=False)
    owT = nc.declare_dram_parameter("owT", [128, HC * HIDDEN], bf16, isOutput=False)
    cos2 = nc.declare_dram_parameter("cos2", [128, S], bf16, isOutput=False)
    sin2 = nc.declare_dram_parameter("sin2", [128, S], bf16, isOutput=False)
    csLp = nc.declare_dram_parameter("csLp", [128, LSL], bf16, isOutput=False)
    snLp = nc.declare_dram_parameter("snLp", [128, LSL], bf16, isOutput=False)
    permT = nc.declare_dram_parameter("permT", [128, 128], bf16, isOutput=False)
    tri01 = nc.declare_dram_parameter("tri01", [128, 128], bf16, isOutput=False)
    out = nc.declare_dram_parameter("out", [S, HIDDEN], bf16, isOutput=True)

    with tile.TileContext(nc) as tc, ExitStack() as ctx:
        p1 = ctx.enter_context(tc.tile_pool(name="p1", bufs=1))
        p2 = ctx.enter_context(tc.tile_pool(name="p2", bufs=2))
        prow = ctx.enter_context(tc.tile_pool(name="prow", bufs=1))
        ph = ctx.enter_context(tc.tile_pool(name="ph", bufs=6))
        pA = ctx.enter_context(tc.tile_pool(name="pA", bufs=4))
        pCh = ctx.enter_context(tc.tile_pool(name="pCh", bufs=4, space="PSUM"))
        pAcc = ctx.enter_context(tc.tile_pool(name="pAcc", bufs=2, space="PSUM"))
        pRow = ctx.enter_context(tc.tile_pool(name="pRow", bufs=1, space="PSUM"))
        pBc = ctx.enter_context(tc.tile_pool(name="pBc", bufs=1, space="PSUM"))
        dram = ctx.enter_context(tc.tile_pool(name="dram", bufs=1, space="DRAM"))

        # ---- constants ----
        ones_col = p1.tile([128, 1], bf16, name="ones_col", tag="ones_col")
        nc.vector.memset(ones_col, 1.0)
        # all-ones rows at partitions 0 and 32 (lhsT for rank-1 broadcasts;
        # partition 32 serves the second head of an attention pair)
        ones_row = p1.tile([33, 128], bf16, name="ones_row", tag="ones_row")
        nc.vector.memset(ones_row, 1.0)
        eps_t = p1.tile([1, 1], f32, name="eps", tag="eps")
        nc.vector.memset(eps_t, LN_EPS)
        perm_sb = p1.tile([128, 128], bf16, name="perm", tag="perm")
        nc.sync.dma_start(out=perm_sb, in_=permT[:, :])
        tri_sb = p1.tile([128, 128], bf16, name="tri", tag="tri")
        nc.sync.dma_start(out=tri_sb, in_=tri01[:, :])

        # ---- persistent activations ----
        qn = [p1.tile([128, S], bf16, name=f"qn{h}", tag=f"qn{h}") for h in range(HC)]
        qpeR = [p1.tile([128, S], bf16, name=f"qpeR{m}", tag=f"qpeR{m}") for m in range(2)]
        # lnT in phases C (gathered); aliased as outT (per head) in phases D/E
        lnT = [p1.tile([128, S], bf16, name=f"lnT{t}", tag=f"lnT{t}") for t in range(4)]
        kpeRd = p1.tile([128, S], bf16, name="kpeRd", tag="kpeRd")

        # ---- big weight buffers (manually phase-shared regions) ----
        # wbuf1: qwT (48KB/part) in B -> owT (32KB) + kn (16KB) after
        wbuf1 = p1.tile([128, KT * QW_COLS], bf16, name="wbuf1", tag="wbuf1")
        # wbuf2: kvawT (40KB/part) in L -> kbw (8KB) + V (16KB) after
        wbuf2 = p1.tile([128, KT * KVA_COLS], bf16, name="wbuf2", tag="wbuf2")
        # Startup DMA plan. The SDMA engines round-robin all queued work at
        # packet granularity, so only per-ring FIFO order gives priority:
        #   scalar ring: hL halves, csL/snL, qw chunks   (phase L + B weights)
        #   sync ring:   kvaw chunks, all four hN n=0 pieces
        # Phases L and B consume hidden-state pieces piece-outer (parallel
        # PSUM chains) so compute starts as soon as the first ~2MB lands.
        lpieces = []
        for pc in range(2):
            hp = ph.tile([128, 16 * LSL], bf16, name="hlp", tag="hp")
            nc.scalar.dma_start(out=hp, in_=hL[pc])
            lpieces.append(hp)
        csL = p1.tile([128, LSL], bf16, name="csL", tag="csL")
        snL = p1.tile([128, LSL], bf16, name="snL", tag="snL")
        nc.scalar.dma_start(out=csL, in_=csLp[:, :])
        nc.scalar.dma_start(out=snL, in_=snLp[:, :])
        for kc in range(0, KT, 8):
            nc.sync.dma_start(out=wbuf2[:, kc * KVA_COLS:(kc + 8) * KVA_COLS],
                              in_=kvawT[:, kc * KVA_COLS:(kc + 8) * KVA_COLS])
        for kc in range(0, KT, 8):
            nc.scalar.dma_start(out=wbuf1[:, kc * QW_COLS:(kc + 8) * QW_COLS],
                              in_=qwT[:, kc * QW_COLS:(kc + 8) * QW_COLS])
        n0_pieces = []
        for pc in range(NPC):
            hp = ph.tile([128, KP * NSL], bf16, name="hp", tag="hp")
            nc.sync.dma_start(out=hp, in_=hN[0, pc])
            n0_pieces.append(hp)

        def qw_sl(k, c0, w):  # qw k-tile slice [128, w]
            return wbuf1[:, k * QW_COLS + c0: k * QW_COLS + c0 + w]

        def kva_sl(k, c0, w):
            return wbuf2[:, k * KVA_COLS + c0: k * KVA_COLS + c0 + w]

        OW_OFF = 0                     # owT: [128, 4*4096] = 32KB (wbuf1)
        KN_OFF = HC * HIDDEN           # kn: 4 heads x [128, 2048] = 16KB (wbuf1)
        KBW_OFF = 0                    # kbw: [128, 4*1024] = 8KB (wbuf2)
        V_OFF = 4 * 1024               # V: [128, 16*512] = 16KB (wbuf2)

        def kn_sl(h, c0, w):
            return wbuf1[:, KN_OFF + h * S + c0: KN_OFF + h * S + c0 + w]

        def ow_sl(h, c0, w):
            return wbuf1[:, OW_OFF + h * HIDDEN + c0: OW_OFF + h * HIDDEN + c0 + w]

        def kbw_sl(k, c0, w):
            return wbuf2[:, KBW_OFF + k * 1024 + c0: KBW_OFF + k * 1024 + c0 + w]

        def v_sl(blk, c0, w):  # blk in 0..15 over sk, cols = 4 heads * 128 dv
            return wbuf2[:, V_OFF + blk * 512 + c0: V_OFF + blk * 512 + c0 + w]

        def outt_sl(h, c0, w):  # aliases lnT[h]
            return lnT[h][:, c0:c0 + w]

        def rope_apply(dsl, raw, cs, sn, w):
            """dsl = raw*cos + (P@raw)*sin (signs folded into sin)."""
            ps_r = pBc.tile([128, NSL], f32, name="ps_rope", tag="bc")
            nc.tensor.matmul(ps_r[:, :w], perm_sb, raw, start=True, stop=True)
            nc.vector.tensor_mul(dsl, raw, cs)
            t1 = p2.tile([128, NSL], bf16, name="rope_t1", tag="rope_t1")
            nc.vector.tensor_mul(t1[:, :w], ps_r[:, :w], sn)
            nc.vector.tensor_add(dsl, dsl, t1[:, :w])

        # ============ Phase L: sharded latent + LN + k-rope + AllGather ============
        in_b = dram.tile([128, 5 * LSL], bf16, name="in_b", tag="in_b")
        out_b = dram.tile([NCORES, 128, 5 * LSL], bf16, name="out_b", tag="out_b",
                          addr_space="Shared")

        with nc.named_scope("phL"):
            lnL = p1.tile([128, 5 * LSL], bf16, name="lnL", tag="lnL")
            # piece-outer: 5 parallel PSUM chains consume each 1MB hL half as
            # soon as it lands (4 banks from pCh + 1 from the idle pAcc)
            ps_l = [pCh.tile([128, NSL], f32, name=f"psL{m}", tag="chain")
                    for m in range(4)]
            ps_l.append(pAcc.tile([128, NSL], f32, name="psL4", tag="acc"))
            for pc in range(2):
                for m in range(5):
                    for kk in range(16):
                        k = pc * 16 + kk
                        nc.tensor.matmul(
                            ps_l[m][:, :LSL], kva_sl(k, m * 128, 128),
                            lpieces[pc][:, kk * LSL:(kk + 1) * LSL],
                            start=(k == 0), stop=(k == KT - 1))
            for m in range(4):
                nc.scalar.activation(out=lnL[:, m * LSL:(m + 1) * LSL],
                                     in_=ps_l[m][:, :LSL], func=Copy)
            kpe_raw = p2.tile([128, LSL], bf16, name="kpe_raw", tag="praw")
            nc.scalar.activation(out=kpe_raw, in_=ps_l[4][:, :LSL], func=Copy)
            rope_apply(lnL[:, 4 * LSL:5 * LSL], kpe_raw, csL, snL, LSL)

            # LN stats + apply on the 4 latent ranges of lnL
            r_mu = prow.tile([1, LSL], f32, name="r_mu", tag="r_mu")
            r_e2 = prow.tile([1, LSL], f32, name="r_e2", tag="r_e2")
            ps1 = pRow.tile([1, NSL], f32, name="ps1", tag="row")
            for t in range(4):
                nc.tensor.matmul(ps1[:, :LSL], ones_col, lnL[:, t * LSL:(t + 1) * LSL],
                                 start=(t == 0), stop=(t == 3))
            nc.vector.tensor_scalar_mul(r_mu, ps1[:, :LSL], 1.0 / KV_LORA)
            ps2 = pRow.tile([1, NSL], f32, name="ps2", tag="row")
            for t in range(4):
                x2 = p2.tile([128, LSL], bf16, name="x2", tag="x2")
                nc.vector.tensor_mul(x2, lnL[:, t * LSL:(t + 1) * LSL],
                                     lnL[:, t * LSL:(t + 1) * LSL])
                nc.tensor.matmul(ps2[:, :LSL], ones_col, x2, start=(t == 0), stop=(t == 3))
            nc.vector.tensor_scalar_mul(r_e2, ps2[:, :LSL], 1.0 / KV_LORA)
            r_m2 = prow.tile([1, LSL], f32, name="r_m2", tag="r_m2")
            nc.vector.tensor_mul(r_m2, r_mu, r_mu)
            nc.vector.tensor_sub(r_e2, r_e2, r_m2)
            nc.scalar.activation(out=r_e2, in_=r_e2, func=Sqrt, bias=eps_t)
            nc.vector.reciprocal(r_m2, r_e2)          # rstd
            r_rb = prow.tile([1, LSL], bf16, name="r_rb", tag="r_rb")
            nc.vector.tensor_copy(r_rb, r_m2)
            nc.vector.tensor_mul(r_mu, r_mu, r_m2)    # mu*rstd
            r_bb = prow.tile([1, LSL], bf16, name="r_bb", tag="r_bb")
            nc.vector.tensor_copy(r_bb, r_mu)
            bc1 = pBc.tile([128, NSL], f32, name="bc1", tag="bc")
            nc.tensor.matmul(bc1[:, :LSL], ones_row[0:1, :], r_rb, start=True, stop=True)
            for t in range(4):
                nc.vector.tensor_mul(lnL[:, t * LSL:(t + 1) * LSL],
                                     lnL[:, t * LSL:(t + 1) * LSL], bc1[:, :LSL])
            bc2 = pBc.tile([128, NSL], f32, name="bc2", tag="bc")
            nc.tensor.matmul(bc2[:, :LSL], ones_row[0:1, :], r_bb, start=True, stop=True)
            for t in range(4):
                nc.vector.tensor_sub(lnL[:, t * LSL:(t + 1) * LSL],
                                     lnL[:, t * LSL:(t + 1) * LSL], bc2[:, :LSL])

            # pack -> all-gather (gpsimd queue keeps straight-line ordering)
            nc.gpsimd.dma_start(out=in_b, in_=lnL)
            nc.gpsimd.collective_compute(
                "AllGather", mybir.AluOpType.bypass,
                replica_groups=[list(range(NCORES))],
                ins=[in_b.opt()], outs=[out_b.opt()])
            # NOTE: the unpack DMAs are emitted AFTER phase B (below) — they
            # block on the collective, and a blocked dma_start stalls its
            # issuing engine's whole queue (head-of-line), which would starve
            # phase B's hN loads.

        # =================== Phase B: q projections ===================
        # piece-outer: 6 parallel PSUM chains (4 nope + 2 rope) consume each
        # 2MB hN piece as it lands; pieces stream on alternating rings.
        with nc.named_scope("phB"):
            for n in range(NN):
                sl0 = n * NSL
                if n == 0:
                    pieces = n0_pieces
                else:
                    eng = nc.scalar if n % 2 == 1 else nc.sync
                    pieces = []
                    for pc in range(NPC):
                        hp = ph.tile([128, KP * NSL], bf16, name="hp", tag="hp")
                        eng.dma_start(out=hp, in_=hN[n, pc])
                        pieces.append(hp)
                cs = p2.tile([128, NSL], bf16, name="cs", tag="cs")
                sn = p2.tile([128, NSL], bf16, name="sn", tag="sn")
                nc.scalar.dma_start(out=cs, in_=cos2[:, sl0:sl0 + NSL])
                nc.scalar.dma_start(out=sn, in_=sin2[:, sl0:sl0 + NSL])

                ps_b = [pCh.tile([128, NSL], f32, name=f"psB{m}", tag="chain")
                        for m in range(4)]
                ps_b += [pAcc.tile([128, NSL], f32, name=f"psB{m+4}", tag="acc")
                         for m in range(2)]
                for pc in range(NPC):
                    for m in range(6):
                        for kk in range(KP):
                            k = pc * KP + kk
                            nc.tensor.matmul(
                                ps_b[m], qw_sl(k, m * 128, 128),
                                pieces[pc][:, kk * NSL:(kk + 1) * NSL],
                                start=(k == 0), stop=(k == KT - 1))
                for h in range(HC):
                    nc.scalar.activation(out=qn[h][:, sl0:sl0 + NSL], in_=ps_b[h],
                                         func=Copy)
                for m in range(2):
                    q_raw = p2.tile([128, NSL], bf16, name="q_raw", tag="praw")
                    nc.scalar.activation(out=q_raw, in_=ps_b[4 + m], func=Copy)
                    rope_apply(qpeR[m][:, sl0:sl0 + NSL], q_raw, cs, sn, NSL)

        # kbw: wbuf2 WAR clears early (end of phase L chains) -> sync queue.
        # owT: wbuf1 WAR clears only when phase B finishes reading qwT, so a
        # blocked dma_start would stall its queue -> park it on gpsimd (idle
        # after the collective).
        nc.sync.dma_start(out=kbw_sl(0, 0, 4 * 1024), in_=kbw[:, :])
        nc.gpsimd.dma_start(out=wbuf1[:, OW_OFF:OW_OFF + HC * HIDDEN], in_=owT[:, :])

        # unpack every core's slice into lnT / kpeRd (after phase B emission
        # so these collective-blocked DMAs sit at the back of the queues)
        with nc.named_scope("phU"):
            for c8 in range(NCORES):
                for t in range(4):
                    eng = nc.sync if (c8 + t) % 2 == 0 else nc.scalar
                    eng.dma_start(out=lnT[t][:, c8 * LSL:(c8 + 1) * LSL],
                                  in_=out_b[c8, :, t * LSL:(t + 1) * LSL])
                eng = nc.sync if c8 % 2 == 0 else nc.scalar
                eng.dma_start(out=kpeRd[:, c8 * LSL:(c8 + 1) * LSL],
                              in_=out_b[c8, :, 4 * LSL:5 * LSL])

        # =================== Phase C: kv_b projections ===================
        with nc.named_scope("phC"):

            for nj in range(NN):
                sl0 = nj * NSL
                for h in range(HC):
                    ps = pCh.tile([128, NSL], f32, name="psKn", tag="chain")
                    for k in range(4):
                        nc.tensor.matmul(ps, kbw_sl(k, h * 128, 128),
                                         lnT[k][:, sl0:sl0 + NSL],
                                         start=(k == 0), stop=(k == 3))
                    nc.scalar.activation(out=kn_sl(h, sl0, NSL), in_=ps, func=Copy)
                for b in range(4):
                    blk = nj * 4 + b
                    ps = pCh.tile([128, NSL], f32, name="psV", tag="chain")
                    for k in range(4):
                        nc.tensor.matmul(ps, lnT[k][:, blk * 128:(blk + 1) * 128],
                                         kbw_sl(k, 512, 512),
                                         start=(k == 0), stop=(k == 3))
                    nc.scalar.activation(out=v_sl(blk, 0, 512), in_=ps, func=Copy)

        # =================== Phase D: attention ===================
        # Heads are processed in pairs (h0 at rows 0-63, h1 at rows 64-127 of
        # the duplicated kpe / stacked q_pe): emitting the two K=64 rope-score
        # matmuls back-to-back lets them run concurrently in disjoint row
        # groups of the PE array.
        def attn_pair(h0, nj):
            sl0 = nj * NSL
            nb = 4 * (nj + 1)
            hs = (h0, h0 + 1)
            qpt = qpeR[h0 // 2]
            ps_o = [pAcc.tile([128, NSL], f32, name=f"ps_o{j}", tag="acc")
                    for j in range(2)]
            # both heads' softmax sums share one PSUM bank (partitions 0 / 32,
            # distinct col groups -> the two ones-matmuls can overlap)
            ps_sum2 = pRow.tile([33, NSL], f32, name="ps_sum2", tag="row")
            ps_sum = [ps_sum2[0:1, :], ps_sum2[32:33, :]]
            for i in range(nb):
                d = i - 4 * nj
                ps_s = [pCh.tile([128, NSL], f32, name=f"ps_s{j}", tag="chain")
                        for j in range(2)]
                for j in range(2):
                    nc.tensor.matmul(ps_s[j], kn_sl(hs[j], i * 128, 128),
                                     qn[hs[j]][:, sl0:sl0 + NSL],
                                     start=True, stop=False)
                for j in range(2):  # adjacent K=64 MMs in disjoint row groups
                    half = 64 * j
                    nc.tensor.matmul(ps_s[j],
                                     kpeRd[half:half + 64, i * 128:(i + 1) * 128],
                                     qpt[half:half + 64, sl0:sl0 + NSL],
                                     start=False, stop=True)
                A_pair = []
                for j in range(2):
                    A_t = pA.tile([128, NSL], bf16, name="A", tag="A")
                    if d < 0:
                        nc.scalar.activation(out=A_t, in_=ps_s[j], func=Exp)
                    else:
                        if d > 0:
                            nc.vector.memset(A_t[:, 0:d * 128], 0.0)
                        nc.scalar.activation(out=A_t[:, d * 128:NSL],
                                             in_=ps_s[j][:, d * 128:NSL], func=Exp)
                        nc.vector.tensor_mul(A_t[:, d * 128:(d + 1) * 128],
                                             A_t[:, d * 128:(d + 1) * 128], tri_sb)
                    A_pair.append(A_t)
                for j in range(2):  # adjacent: col groups 0/32 can overlap
                    nc.tensor.matmul(ps_sum[j], ones_col, A_pair[j],
                                     start=(i == 0), stop=(i == nb - 1))
                for j in range(2):
                    nc.tensor.matmul(ps_o[j], v_sl(i, hs[j] * 128, 128), A_pair[j],
                                     start=(i == 0), stop=(i == nb - 1))
            # normalization closure, emitted one pair late (software
            # pipelining) so its broadcast matmul — which waits on DVE —
            # never heads the in-order PE queue while the next pair computes
            def normalize():
                r_s = p2.tile([33, NSL], bf16, name="r_s", tag="r_s")
                for j in range(2):
                    p0 = 32 * j
                    nc.vector.tensor_copy(r_s[p0:p0 + 1, :], ps_sum[j])
                    bc_r = pBc.tile([128, NSL], f32, name="bc_r", tag="bc")
                    nc.tensor.matmul(bc_r, ones_row[p0:p0 + 1, :],
                                     r_s[p0:p0 + 1, :], start=True, stop=True)
                    rec = p2.tile([128, NSL], bf16, name="rec", tag="rec")
                    with nc.allow_low_precision(reason="softmax denom bf16"):
                        nc.vector.reciprocal(rec, bc_r)
                    raw_o = p2.tile([128, NSL], bf16, name="raw_o", tag="raw_o")
                    nc.vector.tensor_copy(raw_o, ps_o[j])  # DVE: ACT busy w/ exp
                    nc.vector.tensor_mul(outt_sl(hs[j], sl0, NSL), raw_o, rec)
            return normalize

        with nc.named_scope("phD"):
            pending = None
            for nj in range(NN):
                for h0 in (0, 2):
                    norm = attn_pair(h0, nj)
                    if pending is not None:
                        pending()
                    pending = norm
            pending()

        # =================== Phase E: partial o_proj ===================
        with nc.named_scope("phE"):
            for ms in range(16):
                ostg = ph.tile([128, HIDDEN], bf16, name="ostg", tag="hp")
                for g in range(2):  # ns groups of 4 (uses 4 PSUM banks)
                    pss = [pCh.tile([128, NSL], f32, name="psE", tag="chain")
                           for _ in range(4)]
                    for hh in range(HC):
                        for q in range(4):
                            ns = g * 4 + q
                            nc.tensor.matmul(
                                pss[q], outt_sl(hh, ms * 128, 128),
                                ow_sl(hh, ns * NSL, NSL),
                                start=(hh == 0), stop=(hh == HC - 1))
                    for q in range(4):
                        ns = g * 4 + q
                        dst = ostg[:, ns * NSL:(ns + 1) * NSL]
                        if q % 2 == 0:
                            nc.vector.tensor_copy(dst, pss[q])
                        else:
                            nc.scalar.activation(out=dst, in_=pss[q], func=Copy)
                nc.sync.dma_start(out=out[ms * 128:(ms + 1) * 128, :], in_=ostg)

    nc.finalize()
    return nc


def _xi_perm():
    # xi = concat(x[0::2], x[1::2]) -> row j of xi is original row perm[j]
    return np.concatenate([np.arange(0, D_ROPE, 2), np.arange(1, D_ROPE, 2)])


def _host_prep(inputs):
    """Build per-core input maps. Returns list of dicts."""
    hidden = np.asarray(inputs["hidden_states"], np.float32)[0]  # [S, HIDDEN]
    q_w = np.asarray(inputs["q_w"], np.float32)
    kv_a_w = np.asarray(inputs["kv_a_w"], np.float32)
    ln_g = np.asarray(inputs["ln_g"], np.float32)
    kv_b_w = np.asarray(inputs["kv_b_w"], np.float32)
    o_w = np.asarray(inputs["o_w"], np.float32)
    cos_c = np.asarray(inputs["cos_cached"], np.float32)
    sin_c = np.asarray(inputs["sin_cached"], np.float32)
    pos = np.asarray(inputs["position_ids"])[0].astype(np.int64)

    perm = _xi_perm()
    cos_p = cos_c[pos]  # [S, 64]
    sin_p = sin_c[pos]
    cosT = np.ascontiguousarray(cos_p.T)  # [64, S]
    sinT = np.ascontiguousarray(sin_p.T)
    sinTs = sinT.copy()
    sinTs[0:32] = -sinTs[0:32]
    cos2 = np.ascontiguousarray(np.concatenate([cosT, cosT], 0).astype(BF16))
    sin2 = np.ascontiguousarray(np.concatenate([sinTs, sinTs], 0).astype(BF16))

    # hN: [NN, NPC, 128, KP*NSL]; hN[n,pc,p,(kk,c)] = hidden[n*NSL+c, (pc*KP+kk)*128+p]
    hT = hidden.T.astype(BF16)  # [HIDDEN, S]
    hN = (hT.reshape(NPC, KP, 128, NN, NSL)
            .transpose(3, 0, 2, 1, 4)
            .reshape(NN, NPC, 128, KP * NSL))
    hN = np.ascontiguousarray(hN)

    # rotate-half permutation: out = P @ x with P = blockdiag(P64, P64),
    # P64[j, (j+32) % 64] = 1  (signs folded into sin2); lhsT = P.T
    P64 = np.zeros((64, 64), np.float32)
    for j in range(64):
        P64[j, (j + 32) % 64] = 1.0
    P128 = np.zeros((128, 128), np.float32)
    P128[:64, :64] = P64
    P128[64:, 64:] = P64
    permT_np = np.ascontiguousarray(P128.T.astype(BF16))

    tri = (np.arange(128)[:, None] <= np.arange(128)[None, :]).astype(np.float32)
    tri01_np = np.ascontiguousarray(tri.astype(BF16))

    kvb_folded = kv_b_w * ln_g[None, :]
    qw_s = q_w * SCALE

    # kvaw extended: 512 latent + rope(perm) duplicated to 128 rows
    kva_rope = kv_a_w[KV_LORA:][perm]  # [64, HIDDEN]
    kvaw_ext = np.concatenate([kv_a_w[:KV_LORA], kva_rope, kva_rope], 0)  # [640, HIDDEN]
    kvawT_p = np.ascontiguousarray(
        kvaw_ext.T.reshape(KT, 128, KVA_COLS).transpose(1, 0, 2)
        .reshape(128, KT * KVA_COLS).astype(BF16))

    in_maps = []
    for c in range(NCORES):
        heads = list(range(c * HC, (c + 1) * HC))
        nope_rows = np.concatenate([np.arange(h * Q_HEAD, h * Q_HEAD + D_NOPE) for h in heads])
        rope_rows = np.concatenate([h * Q_HEAD + D_NOPE + perm for h in heads])
        qw_c = qw_s[np.concatenate([nope_rows, rope_rows])]  # [768, HIDDEN]
        qwT_p = np.ascontiguousarray(
            qw_c.T.reshape(KT, 128, QW_COLS).transpose(1, 0, 2)
            .reshape(128, KT * QW_COLS).astype(BF16))

        knope_rows = np.concatenate([np.arange(h * 256, h * 256 + D_NOPE) for h in heads])
        v_rows = np.concatenate([np.arange(h * 256 + D_NOPE, (h + 1) * 256) for h in heads])
        kbw_c = np.concatenate([kvb_folded[knope_rows], kvb_folded[v_rows]], 0)  # [1024, 512]
        kbw_p = np.ascontiguousarray(
            kbw_c.T.reshape(4, 128, 1024).transpose(1, 0, 2)
            .reshape(128, 4 * 1024).astype(BF16))

        ow_c = o_w[:, c * HC * D_V:(c + 1) * HC * D_V]  # [HIDDEN, 512]
        owT_p = np.ascontiguousarray(
            ow_c.T.reshape(4, 128, HIDDEN).transpose(1, 0, 2)
            .reshape(128, 4 * HIDDEN).astype(BF16))

        # per-core latent slice of hidden: columns [c*LSL, (c+1)*LSL)
        hL_c = (hT[:, c * LSL:(c + 1) * LSL]
                .reshape(2, 16, 128, LSL)
                .transpose(0, 2, 1, 3)
                .reshape(2, 128, 16 * LSL))
        hL_c = np.ascontiguousarray(hL_c)
        csL = np.ascontiguousarray(cos2[:, c * LSL:(c + 1) * LSL])
        snL = np.ascontiguousarray(sin2[:, c * LSL:(c + 1) * LSL])

        m = {"hN": hN, "hL": hL_c, "qwT": qwT_p, "kvawT": kvawT_p, "kbw": kbw_p,
             "owT": owT_p, "cos2": cos2, "sin2": sin2, "csLp": csL, "snLp": snL,
             "permT": permT_np, "tri01": tri01_np}
        in_maps.append(m)
    return in_maps


def _mask_is_causal(mask):
    m = np.asarray(mask, np.float32)[0, 0]
    tri = np.tril(np.ones((S, S), bool))
    return m.shape == (S, S) and np.all(m[tri] == 0.0) and np.all(m[~tri] <= -1e8)


def _numpy_fallback(inputs):
    hs = np.asarray(inputs["hidden_states"], np.float32)
    mask = np.asarray(inputs["attention_mask"], np.float32)
    pos = np.asarray(inputs["position_ids"]).astype(np.int64)
    q_w = np.asarray(inputs["q_w"], np.float32)
    kv_a_w = np.asarray(inputs["kv_a_w"], np.float32)
    ln_g = np.asarray(inputs["ln_g"], np.float32)
    ln_b = np.asarray(inputs["ln_b"], np.float32)
    kv_b_w = np.asarray(inputs["kv_b_w"], np.float32)
    o_w = np.asarray(inputs["o_w"], np.float32)
    cos_c = np.asarray(inputs["cos_cached"], np.float32)
    sin_c = np.asarray(inputs["sin_cached"], np.float32)
    B, Sq, _ = hs.shape
    q = (hs @ q_w.T).reshape(B, Sq, H, Q_HEAD).transpose(0, 2, 1, 3)
    q_nope, q_pe = q[..., :D_NOPE], q[..., D_NOPE:]
    ckv = hs @ kv_a_w.T
    ckv_l, k_pe = ckv[..., :KV_LORA], ckv[..., KV_LORA:]
    k_pe = k_pe[:, None]
    mu = ckv_l.mean(-1, keepdims=True)
    var = ((ckv_l - mu) ** 2).mean(-1, keepdims=True)
    ln = (ckv_l - mu) / np.sqrt(var + LN_EPS) * ln_g + ln_b
    kv = (ln @ kv_b_w.T).reshape(B, Sq, H, D_NOPE + D_V).transpose(0, 2, 1, 3)
    k_nope, v = kv[..., :D_NOPE], kv[..., D_NOPE:]
    cos = cos_c[pos][:, None]
    sin = sin_c[pos][:, None]

    def rope(x):
        xi = np.concatenate([x[..., 0::2], x[..., 1::2]], -1)
        half = xi.shape[-1] // 2
        rot = np.concatenate([-xi[..., half:], xi[..., :half]], -1)
        return xi * cos + rot * sin

    q_pe, k_pe = rope(q_pe), rope(k_pe)
    query = np.concatenate([q_nope, q_pe], -1)
    key = np.concatenate([k_nope, np.broadcast_to(k_pe, (B, H, Sq, D_ROPE))], -1)
    sc = np.einsum("bhqd,bhkd->bhqk", query, key) * SCALE + mask
    sc = sc - sc.max(-1, keepdims=True)
    a = np.exp(sc)
    a /= a.sum(-1, keepdims=True)
    o = np.einsum("bhqk,bhkd->bhqd", a, v)
    o = o.transpose(0, 2, 1, 3).reshape(B, Sq, H * D_V)
    return (o @ o_w.T).astype(np.float32)


def kernel(**inputs):
    if not _mask_is_causal(inputs["attention_mask"]):
        return _numpy_fallback(inputs)
    pos = np.asarray(inputs["position_ids"])[0].astype(np.int64)
    if pos.shape[0] != S or np.asarray(inputs["hidden_states"]).shape != (1, S, HIDDEN):
        return _numpy_fallback(inputs)
    if np.any(np.asarray(inputs["ln_b"], np.float32) != 0.0):
        return _numpy_fallback(inputs)

    from concourse.bass_utils import run_bass_kernel_spmd

    in_maps = _host_prep(inputs)
    if "prog" not in _CACHE:
        _CACHE["prog"] = _build_program()
    nc = _CACHE["prog"]
    res = run_bass_kernel_spmd(nc, in_maps, core_ids=list(range(NCORES)))
    parts = [np.asarray(res.results[i]["out"], np.float32) for i in range(NCORES)]
    total = np.sum(np.stack(parts, 0), 0, dtype=np.float32)
    return total.reshape(1, S, HIDDEN)


# revision 38
# speedup vs baseline: 1.0335x; 1.0335x over previous
"""MockDeepSeekAttention (MLA) fused kernel for 8 TRN2 NeuronCores.

Sharding: tensor-parallel over heads (4 heads/core) for q/kv_b/attention/
o_proj; the shared kv_a latent projection + LN + k-rope is column-sharded
across the 8 cores and all-gathered on-chip (it would otherwise be
replicated work). The host sums the 8 partial o_proj outputs.

Kernel structure (per core):
  L: latent projection for this core's 256-token column slice + LN +
     k-RoPE, packed and AllGather'd via DRAM bounce buffers. The gather
     completes in the background of phase B.
  B: q projections (nope + rope) for this core's 4 heads, N=512 chains,
     fully SBUF-resident.
  C: kv_b projections (k_nope per head, V blocks) from the gathered latent.
  D: causal attention, scores^T layout; softmax sum via ones-matmul;
     mask via sub-sliced exp + memset + [128,128] triangle multiply.
  E: partial o_proj, h-middle loop order so each stationary tile serves
     4 matmuls; bf16 partial output (host sums in f32).
"""

import sys

if "/opt/trn_rl_repo" not in sys.path:
    sys.path.insert(0, "/opt/trn_rl_repo")

import numpy as np
import ml_dtypes

BF16 = ml_dtypes.bfloat16

# Model dims (hardcoded per problem spec)
H = 32
D_NOPE = 128
D_ROPE = 64
D_V = 128
Q_HEAD = D_NOPE + D_ROPE  # 192
KV_LORA = 512
HIDDEN = 4096
S = 2048
NCORES = 8
HC = H // NCORES  # 4 heads per core
SCALE = 1.0 / Q_HEAD**0.5
LN_EPS = 1e-5

KT = HIDDEN // 128   # 32 k-tiles
NSL = 512            # column tile / attention sq tile
NN = S // NSL        # 4
KP = 8               # k-tiles per DMA piece
NPC = KT // KP       # 4 pieces per column tile
LSL = S // NCORES    # 256: per-core latent column slice

QW_COLS = HC * 128 + HC * 64  # 768
KVA_COLS = 640                # 512 latent + 128 duplicated rope

_CACHE = {}


def _build_program():
    import concourse.bass as bass
    import concourse.tile as tile
    from concourse import mybir, bacc
    from contextlib import ExitStack

    f32 = mybir.dt.float32
    bf16 = mybir.dt.bfloat16
    Exp = mybir.ActivationFunctionType.Exp
    Sqrt = mybir.ActivationFunctionType.Sqrt
    Copy = mybir.ActivationFunctionType.Copy

    nc = bacc.Bacc()
    # DRAM params, already in SBUF (partition-major) layout from the host.
    hN = nc.declare_dram_parameter("hN", [NN, NPC, 128, KP * NSL], bf16, isOutput=False)
    hL = nc.declare_dram_parameter("hL", [2, 128, 16 * LSL], bf16, isOutput=False)
    qwT = nc.declare_dram_parameter("qwT", [128, KT * QW_COLS], bf16, isOutput=False)
    kvawT = nc.declare_dram_parameter("kvawT", [128, KT * KVA_COLS], bf16, isOutput=False)
    kbw = nc.declare_dram_parameter("kbw", [128, 4 * 1024], bf16, isOutput=False)
    owT = nc.declare_dram_parameter("owT", [128, HC * HIDDEN], bf16, isOutput=False)
    cos2 = nc.declare_dram_parameter("cos2", [128, S], bf16, isOutput=False)
    sin2 = nc.declare_dram_parameter("sin2", [128, S], bf16, isOutput=False)
    csLp = nc.declare_dram_parameter("csLp", [128, LSL], bf16, isOutput=False)
    snLp = nc.declare_dram_parameter("snLp", [128, LSL], bf16, isOutput=False)
    permT = nc.declare_dram_parameter("permT", [128, 128], bf16, isOutput=False)
    tri01 = nc.declare_dram_parameter("tri01", [128, 128], bf16, isOutput=False)
    out = nc.declare_dram_parameter("out", [S, HIDDEN], bf16, isOutput=True)

    with tile.TileContext(nc) as tc, ExitStack() as ctx:
        p1 = ctx.enter_context(tc.tile_pool(name="p1", bufs=1))
        p2 = ctx.enter_context(tc.tile_pool(name="p2", bufs=2))
        prow = ctx.enter_context(tc.tile_pool(name="prow", bufs=1))
        ph = ctx.enter_context(tc.tile_pool(name="ph", bufs=6))
        pA = ctx.enter_context(tc.tile_pool(name="pA", bufs=4))
        pCh = ctx.enter_context(tc.tile_pool(name="pCh", bufs=4, space="PSUM"))
        pAcc = ctx.enter_context(tc.tile_pool(name="pAcc", bufs=2, space="PSUM"))
        pRow = ctx.enter_context(tc.tile_pool(name="pRow", bufs=1, space="PSUM"))
        pBc = ctx.enter_context(tc.tile_pool(name="pBc", bufs=1, space="PSUM"))
        dram = ctx.enter_context(tc.tile_pool(name="dram", bufs=1, space="DRAM"))

        # ---- constants ----
        ones_col = p1.tile([128, 1], bf16, name="ones_col", tag="ones_col")
        nc.vector.memset(ones_col, 1.0)
        # all-ones rows at partitions 0 and 32 (lhsT for rank-1 broadcasts;
        # partition 32 serves the second head of an attention pair)
        ones_row = p1.tile([33, 128], bf16, name="ones_row", tag="ones_row")
        nc.vector.memset(ones_row, 1.0)
        eps_t = p1.tile([1, 1], f32, name="eps", tag="eps")
        nc.vector.memset(eps_t, LN_EPS)
        perm_sb = p1.tile([128, 128], bf16, name="perm", tag="perm")
        nc.sync.dma_start(out=perm_sb, in_=permT[:, :])
        tri_sb = p1.tile([128, 128], bf16, name="tri", tag="tri")
        nc.sync.dma_start(out=tri_sb, in_=tri01[:, :])

        # ---- persistent activations ----
        qn = [p1.tile([128, S], bf16, name=f"qn{h}", tag=f"qn{h}") for h in range(HC)]
        qpeR = [p1.tile([128, S], bf16, name=f"qpeR{m}", tag=f"qpeR{m}") for m in range(2)]
        # lnT in phases C (gathered); aliased as outT (per head) in phases D/E
        lnT = [p1.tile([128, S], bf16, name=f"lnT{t}", tag=f"lnT{t}") for t in range(4)]
        kpeRd = p1.tile([128, S], bf16, name="kpeRd", tag="kpeRd")

        # ---- big weight buffers (manually phase-shared regions) ----
        # wbuf1: qwT (48KB/part) in B -> owT (32KB) + kn (16KB) after
        wbuf1 = p1.tile([128, KT * QW_COLS], bf16, name="wbuf1", tag="wbuf1")
        # wbuf2: kvawT (40KB/part) in L -> kbw (8KB) + V (16KB) after
        wbuf2 = p1.tile([128, KT * KVA_COLS], bf16, name="wbuf2", tag="wbuf2")
        # Startup DMA plan. The SDMA engines round-robin all queued work at
        # packet granularity, so only per-ring FIFO order gives priority:
        #   scalar ring: hL halves, csL/snL, qw chunks   (phase L + B weights)
        #   sync ring:   kvaw chunks, all four hN n=0 pieces
        # Phases L and B consume hidden-state pieces piece-outer (parallel
        # PSUM chains) so compute starts as soon as the first ~2MB lands.
        def qw_chunk(kc):
            return (wbuf1[:, kc * QW_COLS:(kc + 8) * QW_COLS],
                    qwT[:, kc * QW_COLS:(kc + 8) * QW_COLS])

        lpieces = []
        for pc in range(2):
            hp = ph.tile([128, 16 * LSL], bf16, name="hlp", tag="hp")
            nc.scalar.dma_start(out=hp, in_=hL[pc])
            lpieces.append(hp)
        csL = p1.tile([128, LSL], bf16, name="csL", tag="csL")
        snL = p1.tile([128, LSL], bf16, name="snL", tag="snL")
        nc.scalar.dma_start(out=csL, in_=csLp[:, :])
        nc.scalar.dma_start(out=snL, in_=snLp[:, :])
        for kc in range(0, KT, 8):
            nc.sync.dma_start(out=wbuf2[:, kc * KVA_COLS:(kc + 8) * KVA_COLS],
                              in_=kvawT[:, kc * KVA_COLS:(kc + 8) * KVA_COLS])
        n0_pieces = [None] * NPC

        def n0p(pc, eng):
            hp = ph.tile([128, KP * NSL], bf16, name="hp", tag="hp")
            eng.dma_start(out=hp, in_=hN[0, pc])
            n0_pieces[pc] = hp

        # scalar ring: qw c0, p0, qw c1, qw c2, p2 ; sync ring: p1, qw c3, p3
        o, i_ = qw_chunk(0)
        nc.scalar.dma_start(out=o, in_=i_)
        n0p(0, nc.scalar)
        n0p(1, nc.sync)
        o, i_ = qw_chunk(8)
        nc.scalar.dma_start(out=o, in_=i_)
        o, i_ = qw_chunk(24)
        nc.sync.dma_start(out=o, in_=i_)
        o, i_ = qw_chunk(16)
        nc.scalar.dma_start(out=o, in_=i_)
        n0p(2, nc.scalar)
        n0p(3, nc.sync)

        def qw_sl(k, c0, w):  # qw k-tile slice [128, w]
            return wbuf1[:, k * QW_COLS + c0: k * QW_COLS + c0 + w]

        def kva_sl(k, c0, w):
            return wbuf2[:, k * KVA_COLS + c0: k * KVA_COLS + c0 + w]

        OW_OFF = 0                     # owT: [128, 4*4096] = 32KB (wbuf1)
        KN_OFF = HC * HIDDEN           # kn: 4 heads x [128, 2048] = 16KB (wbuf1)
        KBW_OFF = 0                    # kbw: [128, 4*1024] = 8KB (wbuf2)
        V_OFF = 4 * 1024               # V: [128, 16*512] = 16KB (wbuf2)

        def kn_sl(h, c0, w):
            return wbuf1[:, KN_OFF + h * S + c0: KN_OFF + h * S + c0 + w]

        def ow_sl(h, c0, w):
            return wbuf1[:, OW_OFF + h * HIDDEN + c0: OW_OFF + h * HIDDEN + c0 + w]

        def kbw_sl(k, c0, w):
            return wbuf2[:, KBW_OFF + k * 1024 + c0: KBW_OFF + k * 1024 + c0 + w]

        def v_sl(blk, c0, w):  # blk in 0..15 over sk, cols = 4 heads * 128 dv
            return wbuf2[:, V_OFF + blk * 512 + c0: V_OFF + blk * 512 + c0 + w]

        def outt_sl(h, c0, w):  # aliases lnT[h]
            return lnT[h][:, c0:c0 + w]

        def rope_apply(dsl, raw, cs, sn, w):
            """dsl = raw*cos + (P@raw)*sin (signs folded into sin)."""
            ps_r = pBc.tile([128, NSL], f32, name="ps_rope", tag="bc")
            nc.tensor.matmul(ps_r[:, :w], perm_sb, raw, start=True, stop=True)
            nc.vector.tensor_mul(dsl, raw, cs)
            t1 = p2.tile([128, NSL], bf16, name="rope_t1", tag="rope_t1")
            nc.vector.tensor_mul(t1[:, :w], ps_r[:, :w], sn)
            nc.vector.tensor_add(dsl, dsl, t1[:, :w])

        # ============ Phase L: sharded latent + LN + k-rope + AllGather ============
        in_b = dram.tile([128, 5 * LSL], bf16, name="in_b", tag="in_b")
        out_b = dram.tile([NCORES, 128, 5 * LSL], bf16, name="out_b", tag="out_b",
                          addr_space="Shared")

        with nc.named_scope("phL"):
            lnL = p1.tile([128, 5 * LSL], bf16, name="lnL", tag="lnL")
            # piece-outer: 5 parallel PSUM chains consume each 1MB hL half as
            # soon as it lands (4 banks from pCh + 1 from the idle pAcc)
            ps_l = [pCh.tile([128, NSL], f32, name=f"psL{m}", tag="chain")
                    for m in range(4)]
            ps_l.append(pAcc.tile([128, NSL], f32, name="psL4", tag="acc"))
            for pc in range(2):
                for m in range(5):
                    for kk in range(16):
                        k = pc * 16 + kk
                        nc.tensor.matmul(
                            ps_l[m][:, :LSL], kva_sl(k, m * 128, 128),
                            lpieces[pc][:, kk * LSL:(kk + 1) * LSL],
                            start=(k == 0), stop=(k == KT - 1))
            for m in range(4):
                nc.scalar.activation(out=lnL[:, m * LSL:(m + 1) * LSL],
                                     in_=ps_l[m][:, :LSL], func=Copy)
            kpe_raw = p2.tile([128, LSL], bf16, name="kpe_raw", tag="praw")
            nc.scalar.activation(out=kpe_raw, in_=ps_l[4][:, :LSL], func=Copy)
            rope_apply(lnL[:, 4 * LSL:5 * LSL], kpe_raw, csL, snL, LSL)

            # LN stats + apply on the 4 latent ranges of lnL
            r_mu = prow.tile([1, LSL], f32, name="r_mu", tag="r_mu")
            r_e2 = prow.tile([1, LSL], f32, name="r_e2", tag="r_e2")
            ps1 = pRow.tile([1, NSL], f32, name="ps1", tag="row")
            for t in range(4):
                nc.tensor.matmul(ps1[:, :LSL], ones_col, lnL[:, t * LSL:(t + 1) * LSL],
                                 start=(t == 0), stop=(t == 3))
            nc.vector.tensor_scalar_mul(r_mu, ps1[:, :LSL], 1.0 / KV_LORA)
            ps2 = pRow.tile([1, NSL], f32, name="ps2", tag="row")
            for t in range(4):
                x2 = p2.tile([128, LSL], bf16, name="x2", tag="x2")
                nc.vector.tensor_mul(x2, lnL[:, t * LSL:(t + 1) * LSL],
                                     lnL[:, t * LSL:(t + 1) * LSL])
                nc.tensor.matmul(ps2[:, :LSL], ones_col, x2, start=(t == 0), stop=(t == 3))
            nc.vector.tensor_scalar_mul(r_e2, ps2[:, :LSL], 1.0 / KV_LORA)
            r_m2 = prow.tile([1, LSL], f32, name="r_m2", tag="r_m2")
            nc.vector.tensor_mul(r_m2, r_mu, r_mu)
            nc.vector.tensor_sub(r_e2, r_e2, r_m2)
            nc.scalar.activation(out=r_e2, in_=r_e2, func=Sqrt, bias=eps_t)
            nc.vector.reciprocal(r_m2, r_e2)          # rstd
            r_rb = prow.tile([1, LSL], bf16, name="r_rb", tag="r_rb")
            nc.vector.tensor_copy(r_rb, r_m2)
            nc.vector.tensor_mul(r_mu, r_mu, r_m2)    # mu*rstd
            r_bb = prow.tile([1, LSL], bf16, name="r_bb", tag="r_bb")
            nc.vector.tensor_copy(r_bb, r_mu)
            bc1 = pBc.tile([128, NSL], f32, name="bc1", tag="bc")
            nc.tensor.matmul(bc1[:, :LSL], ones_row[0:1, :], r_rb, start=True, stop=True)
            for t in range(4):
                nc.vector.tensor_mul(lnL[:, t * LSL:(t + 1) * LSL],
                                     lnL[:, t * LSL:(t + 1) * LSL], bc1[:, :LSL])
            bc2 = pBc.tile([128, NSL], f32, name="bc2", tag="bc")
            nc.tensor.matmul(bc2[:, :LSL], ones_row[0:1, :], r_bb, start=True, stop=True)
            for t in range(4):
                nc.vector.tensor_sub(lnL[:, t * LSL:(t + 1) * LSL],
                                     lnL[:, t * LSL:(t + 1) * LSL], bc2[:, :LSL])

            # pack -> all-gather (gpsimd queue keeps straight-line ordering)
            nc.gpsimd.dma_start(out=in_b, in_=lnL)
            nc.gpsimd.collective_compute(
                "AllGather", mybir.AluOpType.bypass,
                replica_groups=[list(range(NCORES))],
                ins=[in_b.opt()], outs=[out_b.opt()])
            # NOTE: the unpack DMAs are emitted AFTER phase B (below) — they
            # block on the collective, and a blocked dma_start stalls its
            # issuing engine's whole queue (head-of-line), which would starve
            # phase B's hN loads.

        # =================== Phase B: q projections ===================
        # piece-outer: 6 parallel PSUM chains (4 nope + 2 rope) consume each
        # 2MB hN piece as it lands; pieces stream on alternating rings.
        with nc.named_scope("phB"):
            for n in range(NN):
                sl0 = n * NSL
                if n == 0:
                    pieces = n0_pieces
                else:
                    eng = nc.scalar if n % 2 == 1 else nc.sync
                    pieces = []
                    for pc in range(NPC):
                        hp = ph.tile([128, KP * NSL], bf16, name="hp", tag="hp")
                        eng.dma_start(out=hp, in_=hN[n, pc])
                        pieces.append(hp)
                cs = p2.tile([128, NSL], bf16, name="cs", tag="cs")
                sn = p2.tile([128, NSL], bf16, name="sn", tag="sn")
                nc.scalar.dma_start(out=cs, in_=cos2[:, sl0:sl0 + NSL])
                nc.scalar.dma_start(out=sn, in_=sin2[:, sl0:sl0 + NSL])

                ps_b = [pCh.tile([128, NSL], f32, name=f"psB{m}", tag="chain")
                        for m in range(4)]
                ps_b += [pAcc.tile([128, NSL], f32, name=f"psB{m+4}", tag="acc")
                         for m in range(2)]
                for pc in range(NPC):
                    for m in range(6):
                        for kk in range(KP):
                            k = pc * KP + kk
                            nc.tensor.matmul(
                                ps_b[m], qw_sl(k, m * 128, 128),
                                pieces[pc][:, kk * NSL:(kk + 1) * NSL],
                                start=(k == 0), stop=(k == KT - 1))
                for h in range(HC):
                    nc.scalar.activation(out=qn[h][:, sl0:sl0 + NSL], in_=ps_b[h],
                                         func=Copy)
                for m in range(2):
                    q_raw = p2.tile([128, NSL], bf16, name="q_raw", tag="praw")
                    nc.scalar.activation(out=q_raw, in_=ps_b[4 + m], func=Copy)
                    rope_apply(qpeR[m][:, sl0:sl0 + NSL], q_raw, cs, sn, NSL)

        # kbw: wbuf2 WAR clears early (end of phase L chains) -> sync queue.
        # owT: wbuf1 WAR clears only when phase B finishes reading qwT, so a
        # blocked dma_start would stall its queue -> park it on gpsimd (idle
        # after the collective).
        nc.sync.dma_start(out=kbw_sl(0, 0, 4 * 1024), in_=kbw[:, :])
        nc.gpsimd.dma_start(out=wbuf1[:, OW_OFF:OW_OFF + HC * HIDDEN], in_=owT[:, :])

        # unpack every core's slice into lnT / kpeRd (after phase B emission
        # so these collective-blocked DMAs sit at the back of the queues)
        with nc.named_scope("phU"):
            for c8 in range(NCORES):
                for t in range(4):
                    eng = nc.sync if (c8 + t) % 2 == 0 else nc.scalar
                    eng.dma_start(out=lnT[t][:, c8 * LSL:(c8 + 1) * LSL],
                                  in_=out_b[c8, :, t * LSL:(t + 1) * LSL])
                eng = nc.sync if c8 % 2 == 0 else nc.scalar
                eng.dma_start(out=kpeRd[:, c8 * LSL:(c8 + 1) * LSL],
                              in_=out_b[c8, :, 4 * LSL:5 * LSL])

        # =================== Phase C: kv_b projections ===================
        with nc.named_scope("phC"):

            for nj in range(NN):
                sl0 = nj * NSL
                for h in range(HC):
                    ps = pCh.tile([128, NSL], f32, name="psKn", tag="chain")
                    for k in range(4):
                        nc.tensor.matmul(ps, kbw_sl(k, h * 128, 128),
                                         lnT[k][:, sl0:sl0 + NSL],
                                         start=(k == 0), stop=(k == 3))
                    nc.scalar.activation(out=kn_sl(h, sl0, NSL), in_=ps, func=Copy)
                for b in range(4):
                    blk = nj * 4 + b
                    ps = pCh.tile([128, NSL], f32, name="psV", tag="chain")
                    for k in range(4):
                        nc.tensor.matmul(ps, lnT[k][:, blk * 128:(blk + 1) * 128],
                                         kbw_sl(k, 512, 512),
                                         start=(k == 0), stop=(k == 3))
                    nc.scalar.activation(out=v_sl(blk, 0, 512), in_=ps, func=Copy)

        # =================== Phase D: attention ===================
        # Heads are processed in pairs (h0 at rows 0-63, h1 at rows 64-127 of
        # the duplicated kpe / stacked q_pe): emitting the two K=64 rope-score
        # matmuls back-to-back lets them run concurrently in disjoint row
        # groups of the PE array.
        def attn_pair(h0, nj):
            sl0 = nj * NSL
            nb = 4 * (nj + 1)
            hs = (h0, h0 + 1)
            qpt = qpeR[h0 // 2]
            ps_o = [pAcc.tile([128, NSL], f32, name=f"ps_o{j}", tag="acc")
                    for j in range(2)]
            # both heads' softmax sums share one PSUM bank (partitions 0 / 32,
            # distinct col groups -> the two ones-matmuls can overlap)
            ps_sum2 = pRow.tile([33, NSL], f32, name="ps_sum2", tag="row")
            ps_sum = [ps_sum2[0:1, :], ps_sum2[32:33, :]]
            for i in range(nb):
                d = i - 4 * nj
                ps_s = [pCh.tile([128, NSL], f32, name=f"ps_s{j}", tag="chain")
                        for j in range(2)]
                for j in range(2):
                    nc.tensor.matmul(ps_s[j], kn_sl(hs[j], i * 128, 128),
                                     qn[hs[j]][:, sl0:sl0 + NSL],
                                     start=True, stop=False)
                for j in range(2):  # adjacent K=64 MMs in disjoint row groups
                    half = 64 * j
                    nc.tensor.matmul(ps_s[j],
                                     kpeRd[half:half + 64, i * 128:(i + 1) * 128],
                                     qpt[half:half + 64, sl0:sl0 + NSL],
                                     start=False, stop=True)
                A_pair = []
                for j in range(2):
                    A_t = pA.tile([128, NSL], bf16, name="A", tag="A")
                    if d < 0:
                        nc.scalar.activation(out=A_t, in_=ps_s[j], func=Exp)
                    else:
                        if d > 0:
                            nc.vector.memset(A_t[:, 0:d * 128], 0.0)
                        nc.scalar.activation(out=A_t[:, d * 128:NSL],
                                             in_=ps_s[j][:, d * 128:NSL], func=Exp)
                        nc.vector.tensor_mul(A_t[:, d * 128:(d + 1) * 128],
                                             A_t[:, d * 128:(d + 1) * 128], tri_sb)
                    A_pair.append(A_t)
                for j in range(2):  # adjacent: col groups 0/32 can overlap
                    nc.tensor.matmul(ps_sum[j], ones_col, A_pair[j],
                                     start=(i == 0), stop=(i == nb - 1))
                for j in range(2):
                    nc.tensor.matmul(ps_o[j], v_sl(i, hs[j] * 128, 128), A_pair[j],
                                     start=(i == 0), stop=(i == nb - 1))
            # normalization closure, emitted one pair late (software
            # pipelining) so its broadcast matmul — which waits on DVE —
            # never heads the in-order PE queue while the next pair computes
            def normalize():
                r_s = p2.tile([33, NSL], bf16, name="r_s", tag="r_s")
                for j in range(2):
                    p0 = 32 * j
                    nc.vector.tensor_copy(r_s[p0:p0 + 1, :], ps_sum[j])
                    bc_r = pBc.tile([128, NSL], f32, name="bc_r", tag="bc")
                    nc.tensor.matmul(bc_r, ones_row[p0:p0 + 1, :],
                                     r_s[p0:p0 + 1, :], start=True, stop=True)
                    rec = p2.tile([128, NSL], bf16, name="rec", tag="rec")
                    with nc.allow_low_precision(reason="softmax denom bf16"):
                        nc.vector.reciprocal(rec, bc_r)
                    raw_o = p2.tile([128, NSL], bf16, name="raw_o", tag="raw_o")
                    nc.vector.tensor_copy(raw_o, ps_o[j])  # DVE: ACT busy w/ exp
                    nc.vector.tensor_mul(outt_sl(hs[j], sl0, NSL), raw_o, rec)
            return normalize

        with nc.named_scope("phD"):
            pending = None
            for nj in range(NN):
                for h0 in (0, 2):
                    norm = attn_pair(h0, nj)
                    if pending is not None:
                        pending()
                    pending = norm
            pending()

        # =================== Phase E: partial o_proj ===================
        with nc.named_scope("phE"):
            for ms in range(16):
                ostg = ph.tile([128, HIDDEN], bf16, name="ostg", tag="hp")
                for g in range(2):  # ns groups of 4 (uses 4 PSUM banks)
                    pss = [pCh.tile([128, NSL], f32, name="psE", tag="chain")
                           for _ in range(4)]
                    for hh in range(HC):
                        for q in range(4):
                            ns = g * 4 + q
                            nc.tensor.matmul(
                                pss[q], outt_sl(hh, ms * 128, 128),
                                ow_sl(hh, ns * NSL, NSL),
                                start=(hh == 0), stop=(hh == HC - 1))
                    for q in range(4):
                        ns = g * 4 + q
                        dst = ostg[:, ns * NSL:(ns + 1) * NSL]
                        if q % 2 == 0:
                            nc.vector.tensor_copy(dst, pss[q])
                        else:
                            nc.scalar.activation(out=dst, in_=pss[q], func=Copy)
                nc.sync.dma_start(out=out[ms * 128:(ms + 1) * 128, :], in_=ostg)

    nc.finalize()
    return nc


def _xi_perm():
    # xi = concat(x[0::2], x[1::2]) -> row j of xi is original row perm[j]
    return np.concatenate([np.arange(0, D_ROPE, 2), np.arange(1, D_ROPE, 2)])


def _host_prep(inputs):
    """Build per-core input maps. Returns list of dicts."""
    hidden = np.asarray(inputs["hidden_states"], np.float32)[0]  # [S, HIDDEN]
    q_w = np.asarray(inputs["q_w"], np.float32)
    kv_a_w = np.asarray(inputs["kv_a_w"], np.float32)
    ln_g = np.asarray(inputs["ln_g"], np.float32)
    kv_b_w = np.asarray(inputs["kv_b_w"], np.float32)
    o_w = np.asarray(inputs["o_w"], np.float32)
    cos_c = np.asarray(inputs["cos_cached"], np.float32)
    sin_c = np.asarray(inputs["sin_cached"], np.float32)
    pos = np.asarray(inputs["position_ids"])[0].astype(np.int64)

    perm = _xi_perm()
    cos_p = cos_c[pos]  # [S, 64]
    sin_p = sin_c[pos]
    cosT = np.ascontiguousarray(cos_p.T)  # [64, S]
    sinT = np.ascontiguousarray(sin_p.T)
    sinTs = sinT.copy()
    sinTs[0:32] = -sinTs[0:32]
    cos2 = np.ascontiguousarray(np.concatenate([cosT, cosT], 0).astype(BF16))
    sin2 = np.ascontiguousarray(np.concatenate([sinTs, sinTs], 0).astype(BF16))

    # hN: [NN, NPC, 128, KP*NSL]; hN[n,pc,p,(kk,c)] = hidden[n*NSL+c, (pc*KP+kk)*128+p]
    hT = hidden.T.astype(BF16)  # [HIDDEN, S]
    hN = (hT.reshape(NPC, KP, 128, NN, NSL)
            .transpose(3, 0, 2, 1, 4)
            .reshape(NN, NPC, 128, KP * NSL))
    hN = np.ascontiguousarray(hN)

    # rotate-half permutation: out = P @ x with P = blockdiag(P64, P64),
    # P64[j, (j+32) % 64] = 1  (signs folded into sin2); lhsT = P.T
    P64 = np.zeros((64, 64), np.float32)
    for j in range(64):
        P64[j, (j + 32) % 64] = 1.0
    P128 = np.zeros((128, 128), np.float32)
    P128[:64, :64] = P64
    P128[64:, 64:] = P64
    permT_np = np.ascontiguousarray(P128.T.astype(BF16))

    tri = (np.arange(128)[:, None] <= np.arange(128)[None, :]).astype(np.float32)
    tri01_np = np.ascontiguousarray(tri.astype(BF16))

    kvb_folded = kv_b_w * ln_g[None, :]
    qw_s = q_w * SCALE

    # kvaw extended: 512 latent + rope(perm) duplicated to 128 rows
    kva_rope = kv_a_w[KV_LORA:][perm]  # [64, HIDDEN]
    kvaw_ext = np.concatenate([kv_a_w[:KV_LORA], kva_rope, kva_rope], 0)  # [640, HIDDEN]
    kvawT_p = np.ascontiguousarray(
        kvaw_ext.T.reshape(KT, 128, KVA_COLS).transpose(1, 0, 2)
        .reshape(128, KT * KVA_COLS).astype(BF16))

    in_maps = []
    for c in range(NCORES):
        heads = list(range(c * HC, (c + 1) * HC))
        nope_rows = np.concatenate([np.arange(h * Q_HEAD, h * Q_HEAD + D_NOPE) for h in heads])
        rope_rows = np.concatenate([h * Q_HEAD + D_NOPE + perm for h in heads])
        qw_c = qw_s[np.concatenate([nope_rows, rope_rows])]  # [768, HIDDEN]
        qwT_p = np.ascontiguousarray(
            qw_c.T.reshape(KT, 128, QW_COLS).transpose(1, 0, 2)
            .reshape(128, KT * QW_COLS).astype(BF16))

        knope_rows = np.concatenate([np.arange(h * 256, h * 256 + D_NOPE) for h in heads])
        v_rows = np.concatenate([np.arange(h * 256 + D_NOPE, (h + 1) * 256) for h in heads])
        kbw_c = np.concatenate([kvb_folded[knope_rows], kvb_folded[v_rows]], 0)  # [1024, 512]
        kbw_p = np.ascontiguousarray(
            kbw_c.T.reshape(4, 128, 1024).transpose(1, 0, 2)
            .reshape(128, 4 * 1024).astype(BF16))

        ow_c = o_w[:, c * HC * D_V:(c + 1) * HC * D_V]  # [HIDDEN, 512]
        owT_p = np.ascontiguousarray(
            ow_c.T.reshape(4, 128, HIDDEN).transpose(1, 0, 2)
            .reshape(128, 4 * HIDDEN).astype(BF16))

        # per-core latent slice of hidden: columns [c*LSL, (c+1)*LSL)
        hL_c = (hT[:, c * LSL:(c + 1) * LSL]
                .reshape(2, 16, 128, LSL)
                .transpose(0, 2, 1, 3)
                .reshape(2, 128, 16 * LSL))
        hL_c = np.ascontiguousarray(hL_c)
        csL = np.ascontiguousarray(cos2[:, c * LSL:(c + 1) * LSL])
        snL = np.ascontiguousarray(sin2[:, c * LSL:(c + 1) * LSL])

        m = {"hN": hN, "hL": hL_c, "qwT": qwT_p, "kvawT": kvawT_p, "kbw": kbw_p,
             "owT": owT_p, "cos2": cos2, "sin2": sin2, "csLp": csL, "snLp": snL,
             "permT": permT_np, "tri01": tri01_np}
        in_maps.append(m)
    return in_maps


def _mask_is_causal(mask):
    m = np.asarray(mask, np.float32)[0, 0]
    tri = np.tril(np.ones((S, S), bool))
    return m.shape == (S, S) and np.all(m[tri] == 0.0) and np.all(m[~tri] <= -1e8)


def _numpy_fallback(inputs):
    hs = np.asarray(inputs["hidden_states"], np.float32)
    mask = np.asarray(inputs["attention_mask"], np.float32)
    pos = np.asarray(inputs["position_ids"]).astype(np.int64)
    q_w = np.asarray(inputs["q_w"], np.float32)
    kv_a_w = np.asarray(inputs["kv_a_w"], np.float32)
    ln_g = np.asarray(inputs["ln_g"], np.float32)
    ln_b = np.asarray(inputs["ln_b"], np.float32)
    kv_b_w = np.asarray(inputs["kv_b_w"], np.float32)
    o_w = np.asarray(inputs["o_w"], np.float32)
    cos_c = np.asarray(inputs["cos_cached"], np.float32)
    sin_c = np.asarray(inputs["sin_cached"], np.float32)
    B, Sq, _ = hs.shape
    q = (hs @ q_w.T).reshape(B, Sq, H, Q_HEAD).transpose(0, 2, 1, 3)
    q_nope, q_pe = q[..., :D_NOPE], q[..., D_NOPE:]
    ckv = hs @ kv_a_w.T
    ckv_l, k_pe = ckv[..., :KV_LORA], ckv[..., KV_LORA:]
    k_pe = k_pe[:, None]
    mu = ckv_l.mean(-1, keepdims=True)
    var = ((ckv_l - mu) ** 2).mean(-1, keepdims=True)
    ln = (ckv_l - mu) / np.sqrt(var + LN_EPS) * ln_g + ln_b
    kv = (ln @ kv_b_w.T).reshape(B, Sq, H, D_NOPE + D_V).transpose(0, 2, 1, 3)
    k_nope, v = kv[..., :D_NOPE], kv[..., D_NOPE:]
    cos = cos_c[pos][:, None]
    sin = sin_c[pos][:, None]

    def rope(x):
        xi = np.concatenate([x[..., 0::2], x[..., 1::2]], -1)
        half = xi.shape[-1] // 2
        rot = np.concatenate([-xi[..., half:], xi[..., :half]], -1)
        return xi * cos + rot * sin

    q_pe, k_pe = rope(q_pe), rope(k_pe)
    query = np.concatenate([q_nope, q_pe], -1)
    key = np.concatenate([k_nope, np.broadcast_to(k_pe, (B, H, Sq, D_ROPE))], -1)
    sc = np.einsum("bhqd,bhkd->bhqk", query, key) * SCALE + mask
    sc = sc - sc.max(-1, keepdims=True)
    a = np.exp(sc)
    a /= a.sum(-1, keepdims=True)
    o = np.einsum("bhqk,bhkd->bhqd", a, v)
    o = o.transpose(0, 2, 1, 3).reshape(B, Sq, H * D_V)
    return (o @ o_w.T).astype(np.float32)


def kernel(**inputs):
    if not _mask_is_causal(inputs["attention_mask"]):
        return _numpy_fallback(inputs)
    pos = np.asarray(inputs["position_ids"])[0].astype(np.int64)
    if pos.shape[0] != S or np.asarray(inputs["hidden_states"]).shape != (1, S, HIDDEN):
        return _numpy_fallback(inputs)
    if np.any(np.asarray(inputs["ln_b"], np.float32) != 0.0):
        return _numpy_fallback(inputs)

    from concourse.bass_utils import run_bass_kernel_spmd

    in_maps = _host_prep(inputs)
    if "prog" not in _CACHE:
        _CACHE["prog"] = _build_program()
    nc = _CACHE["prog"]
    res = run_bass_kernel_spmd(nc, in_maps, core_ids=list(range(NCORES)))
    parts = [np.asarray(res.results[i]["out"], np.float32) for i in range(NCORES)]
    total = np.sum(np.stack(parts, 0), 0, dtype=np.float32)
    return total.reshape(1, S, HIDDEN)


# revision 42
# speedup vs baseline: 1.0502x; 1.0161x over previous
"""MockDeepSeekAttention (MLA) fused kernel for 8 TRN2 NeuronCores.

Sharding: tensor-parallel over heads (4 heads/core) for q/kv_b/attention/
o_proj; the shared kv_a latent projection + LN + k-rope is column-sharded
across the 8 cores and all-gathered on-chip (it would otherwise be
replicated work). The host sums the 8 partial o_proj outputs.

Kernel structure (per core):
  L: latent projection for this core's 256-token column slice + LN +
     k-RoPE, packed and AllGather'd via DRAM bounce buffers. The gather
     completes in the background of phase B.
  B: q projections (nope + rope) for this core's 4 heads, N=512 chains,
     fully SBUF-resident.
  C: kv_b projections (k_nope per head, V blocks) from the gathered latent.
  D: causal attention, scores^T layout; softmax sum via ones-matmul;
     mask via sub-sliced exp + memset + [128,128] triangle multiply.
  E: partial o_proj, h-middle loop order so each stationary tile serves
     4 matmuls; bf16 partial output (host sums in f32).
"""

import sys

if "/opt/trn_rl_repo" not in sys.path:
    sys.path.insert(0, "/opt/trn_rl_repo")

import numpy as np
import ml_dtypes

BF16 = ml_dtypes.bfloat16

# Model dims (hardcoded per problem spec)
H = 32
D_NOPE = 128
D_ROPE = 64
D_V = 128
Q_HEAD = D_NOPE + D_ROPE  # 192
KV_LORA = 512
HIDDEN = 4096
S = 2048
NCORES = 8
HC = H // NCORES  # 4 heads per core
SCALE = 1.0 / Q_HEAD**0.5
LN_EPS = 1e-5

KT = HIDDEN // 128   # 32 k-tiles
NSL = 512            # column tile / attention sq tile
NN = S // NSL        # 4
KP = 8               # k-tiles per DMA piece
NPC = KT // KP       # 4 pieces per column tile
LSL = S // NCORES    # 256: per-core latent column slice

QW_COLS = HC * 128 + HC * 64  # 768
KVA_COLS = 640                # 512 latent + 128 duplicated rope

_CACHE = {}


def _build_program():
    import concourse.bass as bass
    import concourse.tile as tile
    from concourse import mybir, bacc
    from contextlib import ExitStack

    f32 = mybir.dt.float32
    bf16 = mybir.dt.bfloat16
    Exp = mybir.ActivationFunctionType.Exp
    Sqrt = mybir.ActivationFunctionType.Sqrt
    Copy = mybir.ActivationFunctionType.Copy

    nc = bacc.Bacc()
    # DRAM params, already in SBUF (partition-major) layout from the host.
    hN = nc.declare_dram_parameter("hN", [NN, NPC, 128, KP * NSL], bf16, isOutput=False)
    hL = nc.declare_dram_parameter("hL", [2, 128, 16 * LSL], bf16, isOutput=False)
    qwT = nc.declare_dram_parameter("qwT", [128, KT * QW_COLS], bf16, isOutput=False)
    kvawT = nc.declare_dram_parameter("kvawT", [128, KT * KVA_COLS], bf16, isOutput=False)
    kbw = nc.declare_dram_parameter("kbw", [128, 4 * 1024], bf16, isOutput=False)
    owT = nc.declare_dram_parameter("owT", [128, HC * HIDDEN], bf16, isOutput=False)
    cos2 = nc.declare_dram_parameter("cos2", [128, S], bf16, isOutput=False)
    sin2 = nc.declare_dram_parameter("sin2", [128, S], bf16, isOutput=False)
    csLp = nc.declare_dram_parameter("csLp", [128, LSL], bf16, isOutput=False)
    snLp = nc.declare_dram_parameter("snLp", [128, LSL], bf16, isOutput=False)
    permT = nc.declare_dram_parameter("permT", [128, 128], bf16, isOutput=False)
    tri01 = nc.declare_dram_parameter("tri01", [128, 128], bf16, isOutput=False)
    out = nc.declare_dram_parameter("out", [S, HIDDEN], bf16, isOutput=True)

    with tile.TileContext(nc) as tc, ExitStack() as ctx:
        p1 = ctx.enter_context(tc.tile_pool(name="p1", bufs=1))
        p2 = ctx.enter_context(tc.tile_pool(name="p2", bufs=2))
        prow = ctx.enter_context(tc.tile_pool(name="prow", bufs=1))
        ph = ctx.enter_context(tc.tile_pool(name="ph", bufs=6))
        pA = ctx.enter_context(tc.tile_pool(name="pA", bufs=6))
        pCh = ctx.enter_context(tc.tile_pool(name="pCh", bufs=4, space="PSUM"))
        pAcc = ctx.enter_context(tc.tile_pool(name="pAcc", bufs=2, space="PSUM"))
        pRow = ctx.enter_context(tc.tile_pool(name="pRow", bufs=1, space="PSUM"))
        pBc = ctx.enter_context(tc.tile_pool(name="pBc", bufs=1, space="PSUM"))
        dram = ctx.enter_context(tc.tile_pool(name="dram", bufs=1, space="DRAM"))

        # ---- constants ----
        ones_col = p1.tile([128, 1], bf16, name="ones_col", tag="ones_col")
        nc.vector.memset(ones_col, 1.0)
        # all-ones rows at partitions 0 and 32 (lhsT for rank-1 broadcasts;
        # partition 32 serves the second head of an attention pair)
        ones_row = p1.tile([33, 128], bf16, name="ones_row", tag="ones_row")
        nc.vector.memset(ones_row, 1.0)
        eps_t = p1.tile([1, 1], f32, name="eps", tag="eps")
        nc.vector.memset(eps_t, LN_EPS)
        perm_sb = p1.tile([128, 128], bf16, name="perm", tag="perm")
        nc.sync.dma_start(out=perm_sb, in_=permT[:, :])
        tri_sb = p1.tile([128, 128], bf16, name="tri", tag="tri")
        nc.sync.dma_start(out=tri_sb, in_=tri01[:, :])

        # ---- persistent activations ----
        qn = [p1.tile([128, S], bf16, name=f"qn{h}", tag=f"qn{h}") for h in range(HC)]
        qpeR = [p1.tile([128, S], bf16, name=f"qpeR{m}", tag=f"qpeR{m}") for m in range(2)]
        # lnT in phases C (gathered); aliased as outT (per head) in phases D/E
        lnT = [p1.tile([128, S], bf16, name=f"lnT{t}", tag=f"lnT{t}") for t in range(4)]
        kpeRd = p1.tile([128, S], bf16, name="kpeRd", tag="kpeRd")

        # ---- big weight buffers (manually phase-shared regions) ----
        # wbuf1: qwT (48KB/part) in B -> owT (32KB) + kn (16KB) after
        wbuf1 = p1.tile([128, KT * QW_COLS], bf16, name="wbuf1", tag="wbuf1")
        # wbuf2: kvawT (40KB/part) in L -> kbw (8KB) + V (16KB) after
        wbuf2 = p1.tile([128, KT * KVA_COLS], bf16, name="wbuf2", tag="wbuf2")
        # Startup DMA plan. The SDMA engines round-robin all queued work at
        # packet granularity, so only per-ring FIFO order gives priority:
        #   scalar ring: hL halves, csL/snL, qw chunks   (phase L + B weights)
        #   sync ring:   kvaw chunks, all four hN n=0 pieces
        # Phases L and B consume hidden-state pieces piece-outer (parallel
        # PSUM chains) so compute starts as soon as the first ~2MB lands.
        def qw_chunk(kc):
            return (wbuf1[:, kc * QW_COLS:(kc + 8) * QW_COLS],
                    qwT[:, kc * QW_COLS:(kc + 8) * QW_COLS])

        lpieces = []
        for pc in range(2):
            hp = ph.tile([128, 16 * LSL], bf16, name="hlp", tag="hp")
            nc.scalar.dma_start(out=hp, in_=hL[pc])
            lpieces.append(hp)
        csL = p1.tile([128, LSL], bf16, name="csL", tag="csL")
        snL = p1.tile([128, LSL], bf16, name="snL", tag="snL")
        nc.scalar.dma_start(out=csL, in_=csLp[:, :])
        nc.scalar.dma_start(out=snL, in_=snLp[:, :])
        for kc in range(0, KT, 8):
            nc.sync.dma_start(out=wbuf2[:, kc * KVA_COLS:(kc + 8) * KVA_COLS],
                              in_=kvawT[:, kc * KVA_COLS:(kc + 8) * KVA_COLS])
        n0_pieces = [None] * NPC

        def n0p(pc, eng):
            hp = ph.tile([128, KP * NSL], bf16, name="hp", tag="hp")
            eng.dma_start(out=hp, in_=hN[0, pc])
            n0_pieces[pc] = hp

        # scalar ring: qw c0, p0, qw c1, qw c2, p2 ; sync ring: p1, qw c3, p3
        o, i_ = qw_chunk(0)
        nc.scalar.dma_start(out=o, in_=i_)
        n0p(0, nc.scalar)
        n0p(1, nc.sync)
        o, i_ = qw_chunk(8)
        nc.scalar.dma_start(out=o, in_=i_)
        o, i_ = qw_chunk(24)
        nc.sync.dma_start(out=o, in_=i_)
        o, i_ = qw_chunk(16)
        nc.scalar.dma_start(out=o, in_=i_)
        n0p(2, nc.scalar)
        n0p(3, nc.sync)

        def qw_sl(k, c0, w):  # qw k-tile slice [128, w]
            return wbuf1[:, k * QW_COLS + c0: k * QW_COLS + c0 + w]

        def kva_sl(k, c0, w):
            return wbuf2[:, k * KVA_COLS + c0: k * KVA_COLS + c0 + w]

        OW_OFF = 0                     # owT: [128, 4*4096] = 32KB (wbuf1)
        KN_OFF = HC * HIDDEN           # kn: 4 heads x [128, 2048] = 16KB (wbuf1)
        KBW_OFF = 0                    # kbw: [128, 4*1024] = 8KB (wbuf2)
        V_OFF = 4 * 1024               # V: [128, 16*512] = 16KB (wbuf2)

        def kn_sl(h, c0, w):
            return wbuf1[:, KN_OFF + h * S + c0: KN_OFF + h * S + c0 + w]

        def ow_sl(h, c0, w):
            return wbuf1[:, OW_OFF + h * HIDDEN + c0: OW_OFF + h * HIDDEN + c0 + w]

        def kbw_sl(k, c0, w):
            return wbuf2[:, KBW_OFF + k * 1024 + c0: KBW_OFF + k * 1024 + c0 + w]

        def v_sl(blk, c0, w):  # blk in 0..15 over sk, cols = 4 heads * 128 dv
            return wbuf2[:, V_OFF + blk * 512 + c0: V_OFF + blk * 512 + c0 + w]

        def outt_sl(h, c0, w):  # aliases lnT[h]
            return lnT[h][:, c0:c0 + w]

        def rope_apply(dsl, raw, cs, sn, w):
            """dsl = raw*cos + (P@raw)*sin (signs folded into sin)."""
            ps_r = pBc.tile([128, NSL], f32, name="ps_rope", tag="bc")
            nc.tensor.matmul(ps_r[:, :w], perm_sb, raw, start=True, stop=True)
            nc.vector.tensor_mul(dsl, raw, cs)
            t1 = p2.tile([128, NSL], bf16, name="rope_t1", tag="rope_t1")
            nc.vector.tensor_mul(t1[:, :w], ps_r[:, :w], sn)
            nc.vector.tensor_add(dsl, dsl, t1[:, :w])

        # ============ Phase L: sharded latent + LN + k-rope + AllGather ============
        in_b = dram.tile([128, 5 * LSL], bf16, name="in_b", tag="in_b")
        out_b = dram.tile([NCORES, 128, 5 * LSL], bf16, name="out_b", tag="out_b",
                          addr_space="Shared")

        with nc.named_scope("phL"):
            lnL = p1.tile([128, 5 * LSL], bf16, name="lnL", tag="lnL")
            # piece-outer: 5 parallel PSUM chains consume each 1MB hL half as
            # soon as it lands (4 banks from pCh + 1 from the idle pAcc)
            ps_l = [pCh.tile([128, NSL], f32, name=f"psL{m}", tag="chain")
                    for m in range(4)]
            ps_l.append(pAcc.tile([128, NSL], f32, name="psL4", tag="acc"))
            for pc in range(2):
                for m in range(5):
                    for kk in range(16):
                        k = pc * 16 + kk
                        nc.tensor.matmul(
                            ps_l[m][:, :LSL], kva_sl(k, m * 128, 128),
                            lpieces[pc][:, kk * LSL:(kk + 1) * LSL],
                            start=(k == 0), stop=(k == KT - 1))
            for m in range(4):
                nc.scalar.activation(out=lnL[:, m * LSL:(m + 1) * LSL],
                                     in_=ps_l[m][:, :LSL], func=Copy)
            kpe_raw = p2.tile([128, LSL], bf16, name="kpe_raw", tag="praw")
            nc.scalar.activation(out=kpe_raw, in_=ps_l[4][:, :LSL], func=Copy)
            rope_apply(lnL[:, 4 * LSL:5 * LSL], kpe_raw, csL, snL, LSL)

            # LN stats + apply on the 4 latent ranges of lnL
            r_mu = prow.tile([1, LSL], f32, name="r_mu", tag="r_mu")
            r_e2 = prow.tile([1, LSL], f32, name="r_e2", tag="r_e2")
            ps1 = pRow.tile([1, NSL], f32, name="ps1", tag="row")
            for t in range(4):
                nc.tensor.matmul(ps1[:, :LSL], ones_col, lnL[:, t * LSL:(t + 1) * LSL],
                                 start=(t == 0), stop=(t == 3))
            nc.vector.tensor_scalar_mul(r_mu, ps1[:, :LSL], 1.0 / KV_LORA)
            ps2 = pRow.tile([1, NSL], f32, name="ps2", tag="row")
            for t in range(4):
                x2 = p2.tile([128, LSL], bf16, name="x2", tag="x2")
                nc.vector.tensor_mul(x2, lnL[:, t * LSL:(t + 1) * LSL],
                                     lnL[:, t * LSL:(t + 1) * LSL])
                nc.tensor.matmul(ps2[:, :LSL], ones_col, x2, start=(t == 0), stop=(t == 3))
            nc.vector.tensor_scalar_mul(r_e2, ps2[:, :LSL], 1.0 / KV_LORA)
            r_m2 = prow.tile([1, LSL], f32, name="r_m2", tag="r_m2")
            nc.vector.tensor_mul(r_m2, r_mu, r_mu)
            nc.vector.tensor_sub(r_e2, r_e2, r_m2)
            nc.scalar.activation(out=r_e2, in_=r_e2, func=Sqrt, bias=eps_t)
            nc.vector.reciprocal(r_m2, r_e2)          # rstd
            r_rb = prow.tile([1, LSL], bf16, name="r_rb", tag="r_rb")
            nc.vector.tensor_copy(r_rb, r_m2)
            nc.vector.tensor_mul(r_mu, r_mu, r_m2)    # mu*rstd
            r_bb = prow.tile([1, LSL], bf16, name="r_bb", tag="r_bb")
            nc.vector.tensor_copy(r_bb, r_mu)
            bc1 = pBc.tile([128, NSL], f32, name="bc1", tag="bc")
            nc.tensor.matmul(bc1[:, :LSL], ones_row[0:1, :], r_rb, start=True, stop=True)
            for t in range(4):
                nc.vector.tensor_mul(lnL[:, t * LSL:(t + 1) * LSL],
                                     lnL[:, t * LSL:(t + 1) * LSL], bc1[:, :LSL])
            bc2 = pBc.tile([128, NSL], f32, name="bc2", tag="bc")
            nc.tensor.matmul(bc2[:, :LSL], ones_row[0:1, :], r_bb, start=True, stop=True)
            for t in range(4):
                nc.vector.tensor_sub(lnL[:, t * LSL:(t + 1) * LSL],
                                     lnL[:, t * LSL:(t + 1) * LSL], bc2[:, :LSL])

            # pack -> all-gather (gpsimd queue keeps straight-line ordering)
            nc.gpsimd.dma_start(out=in_b, in_=lnL)
            nc.gpsimd.collective_compute(
                "AllGather", mybir.AluOpType.bypass,
                replica_groups=[list(range(NCORES))],
                ins=[in_b.opt()], outs=[out_b.opt()])
            # NOTE: the unpack DMAs are emitted AFTER phase B (below) — they
            # block on the collective, and a blocked dma_start stalls its
            # issuing engine's whole queue (head-of-line), which would starve
            # phase B's hN loads.

        # =================== Phase B: q projections ===================
        # piece-outer: 6 parallel PSUM chains (4 nope + 2 rope) consume each
        # 2MB hN piece as it lands; pieces stream on alternating rings.
        with nc.named_scope("phB"):
            for n in range(NN):
                sl0 = n * NSL
                if n == 0:
                    pieces = n0_pieces
                else:
                    eng = nc.scalar if n % 2 == 1 else nc.sync
                    pieces = []
                    for pc in range(NPC):
                        hp = ph.tile([128, KP * NSL], bf16, name="hp", tag="hp")
                        eng.dma_start(out=hp, in_=hN[n, pc])
                        pieces.append(hp)
                cs = p2.tile([128, NSL], bf16, name="cs", tag="cs", bufs=1)
                sn = p2.tile([128, NSL], bf16, name="sn", tag="sn", bufs=1)
                nc.scalar.dma_start(out=cs, in_=cos2[:, sl0:sl0 + NSL])
                nc.scalar.dma_start(out=sn, in_=sin2[:, sl0:sl0 + NSL])

                ps_b = [pCh.tile([128, NSL], f32, name=f"psB{m}", tag="chain")
                        for m in range(4)]
                ps_b += [pAcc.tile([128, NSL], f32, name=f"psB{m+4}", tag="acc")
                         for m in range(2)]
                for pc in range(NPC):
                    for m in range(6):
                        for kk in range(KP):
                            k = pc * KP + kk
                            nc.tensor.matmul(
                                ps_b[m], qw_sl(k, m * 128, 128),
                                pieces[pc][:, kk * NSL:(kk + 1) * NSL],
                                start=(k == 0), stop=(k == KT - 1))
                for h in range(HC):
                    nc.scalar.activation(out=qn[h][:, sl0:sl0 + NSL], in_=ps_b[h],
                                         func=Copy)
                for m in range(2):
                    q_raw = p2.tile([128, NSL], bf16, name="q_raw", tag="praw")
                    nc.scalar.activation(out=q_raw, in_=ps_b[4 + m], func=Copy)
                    rope_apply(qpeR[m][:, sl0:sl0 + NSL], q_raw, cs, sn, NSL)

        # kbw: wbuf2 WAR clears early (end of phase L chains) -> sync queue.
        # owT: wbuf1 WAR clears only when phase B finishes reading qwT, so a
        # blocked dma_start would stall its queue -> park it on gpsimd (idle
        # after the collective).
        nc.sync.dma_start(out=kbw_sl(0, 0, 4 * 1024), in_=kbw[:, :])
        nc.gpsimd.dma_start(out=wbuf1[:, OW_OFF:OW_OFF + HC * HIDDEN], in_=owT[:, :])

        # unpack every core's slice into lnT / kpeRd (after phase B emission
        # so these collective-blocked DMAs sit at the back of the queues)
        with nc.named_scope("phU"):
            for c8 in range(NCORES):
                for t in range(4):
                    eng = nc.sync if (c8 + t) % 2 == 0 else nc.scalar
                    eng.dma_start(out=lnT[t][:, c8 * LSL:(c8 + 1) * LSL],
                                  in_=out_b[c8, :, t * LSL:(t + 1) * LSL])
                eng = nc.sync if c8 % 2 == 0 else nc.scalar
                eng.dma_start(out=kpeRd[:, c8 * LSL:(c8 + 1) * LSL],
                              in_=out_b[c8, :, 4 * LSL:5 * LSL])

        # =================== Phase C: kv_b projections ===================
        with nc.named_scope("phC"):

            for nj in range(NN):
                sl0 = nj * NSL
                for h in range(HC):
                    ps = pCh.tile([128, NSL], f32, name="psKn", tag="chain")
                    for k in range(4):
                        nc.tensor.matmul(ps, kbw_sl(k, h * 128, 128),
                                         lnT[k][:, sl0:sl0 + NSL],
                                         start=(k == 0), stop=(k == 3))
                    nc.scalar.activation(out=kn_sl(h, sl0, NSL), in_=ps, func=Copy)
                for b in range(4):
                    blk = nj * 4 + b
                    ps = pCh.tile([128, NSL], f32, name="psV", tag="chain")
                    for k in range(4):
                        nc.tensor.matmul(ps, lnT[k][:, blk * 128:(blk + 1) * 128],
                                         kbw_sl(k, 512, 512),
                                         start=(k == 0), stop=(k == 3))
                    nc.scalar.activation(out=v_sl(blk, 0, 512), in_=ps, func=Copy)

        # =================== Phase D: attention ===================
        # Heads are processed in pairs (h0 at rows 0-63, h1 at rows 64-127 of
        # the duplicated kpe / stacked q_pe): emitting the two K=64 rope-score
        # matmuls back-to-back lets them run concurrently in disjoint row
        # groups of the PE array.
        def attn_pair(h0, nj, inject=None):
            sl0 = nj * NSL
            nb = 4 * (nj + 1)
            hs = (h0, h0 + 1)
            qpt = qpeR[h0 // 2]
            ps_o = [pAcc.tile([128, NSL], f32, name=f"ps_o{j}", tag="acc")
                    for j in range(2)]
            # both heads' softmax sums share one PSUM bank (partitions 0 / 32,
            # distinct col groups -> the two ones-matmuls can overlap)
            ps_sum2 = pRow.tile([33, NSL], f32, name="ps_sum2", tag="row")
            ps_sum = [ps_sum2[0:1, :], ps_sum2[32:33, :]]

            def sum_v(prev_A, i):
                for j in range(2):  # adjacent: col groups 0/32 can overlap
                    nc.tensor.matmul(ps_sum[j], ones_col, prev_A[j],
                                     start=(i == 0), stop=(i == nb - 1))
                for j in range(2):
                    nc.tensor.matmul(ps_o[j], v_sl(i, hs[j] * 128, 128),
                                     prev_A[j], start=(i == 0), stop=(i == nb - 1))

            prev = None
            for i in range(nb):
                d = i - 4 * nj
                ps_s = [pCh.tile([128, NSL], f32, name=f"ps_s{j}", tag="chain")
                        for j in range(2)]
                for j in range(2):
                    nc.tensor.matmul(ps_s[j], kn_sl(hs[j], i * 128, 128),
                                     qn[hs[j]][:, sl0:sl0 + NSL],
                                     start=True, stop=False)
                for j in range(2):  # adjacent K=64 MMs in disjoint row groups
                    half = 64 * j
                    nc.tensor.matmul(ps_s[j],
                                     kpeRd[half:half + 64, i * 128:(i + 1) * 128],
                                     qpt[half:half + 64, sl0:sl0 + NSL],
                                     start=False, stop=True)
                A_pair = []
                for j in range(2):
                    A_t = pA.tile([128, NSL], bf16, name="A", tag="A")
                    if d < 0:
                        nc.scalar.activation(out=A_t, in_=ps_s[j], func=Exp)
                    else:
                        if d > 0:
                            nc.vector.memset(A_t[:, 0:d * 128], 0.0)
                        nc.scalar.activation(out=A_t[:, d * 128:NSL],
                                             in_=ps_s[j][:, d * 128:NSL], func=Exp)
                        nc.vector.tensor_mul(A_t[:, d * 128:(d + 1) * 128],
                                             A_t[:, d * 128:(d + 1) * 128], tri_sb)
                    A_pair.append(A_t)
                if i == 1 and inject is not None:
                    inject()  # previous pair's normalization (deps long met)
                if prev is not None:
                    sum_v(*prev)  # one block late: aligns the in-order PE
                prev = (A_pair, i)  # queue with actual operand readiness
            sum_v(*prev)
            # normalization closure, emitted one pair late (software
            # pipelining) so its broadcast matmul — which waits on DVE —
            # never heads the in-order PE queue while the next pair computes
            def normalize():
                r_s = p2.tile([33, NSL], bf16, name="r_s", tag="r_s")
                for j in range(2):
                    p0 = 32 * j
                    nc.vector.tensor_copy(r_s[p0:p0 + 1, :], ps_sum[j])
                    bc_r = pBc.tile([128, NSL], f32, name="bc_r", tag="bc")
                    nc.tensor.matmul(bc_r, ones_row[p0:p0 + 1, :],
                                     r_s[p0:p0 + 1, :], start=True, stop=True)
                    rec = p2.tile([128, NSL], bf16, name="rec", tag="rec")
                    with nc.allow_low_precision(reason="softmax denom bf16"):
                        nc.vector.reciprocal(rec, bc_r)
                    raw_o = p2.tile([128, NSL], bf16, name="raw_o", tag="raw_o")
                    nc.vector.tensor_copy(raw_o, ps_o[j])  # DVE: ACT busy w/ exp
                    nc.vector.tensor_mul(outt_sl(hs[j], sl0, NSL), raw_o, rec)
            return normalize

        with nc.named_scope("phD"):
            pending = None
            for nj in range(NN):
                for h0 in (0, 2):
                    pending = attn_pair(h0, nj, inject=pending)
            pending()

        # =================== Phase E: partial o_proj ===================
        with nc.named_scope("phE"):
            for ms in range(16):
                ostg = ph.tile([128, HIDDEN], bf16, name="ostg", tag="hp")
                for g in range(2):  # ns groups of 4 (uses 4 PSUM banks)
                    pss = [pCh.tile([128, NSL], f32, name="psE", tag="chain")
                           for _ in range(4)]
                    for hh in range(HC):
                        for q in range(4):
                            ns = g * 4 + q
                            nc.tensor.matmul(
                                pss[q], outt_sl(hh, ms * 128, 128),
                                ow_sl(hh, ns * NSL, NSL),
                                start=(hh == 0), stop=(hh == HC - 1))
                    for q in range(4):
                        ns = g * 4 + q
                        dst = ostg[:, ns * NSL:(ns + 1) * NSL]
                        if q % 2 == 0:
                            nc.vector.tensor_copy(dst, pss[q])
                        else:
                            nc.scalar.activation(out=dst, in_=pss[q], func=Copy)
                nc.sync.dma_start(out=out[ms * 128:(ms + 1) * 128, :], in_=ostg)

    nc.finalize()
    return nc


def _xi_perm():
    # xi = concat(x[0::2], x[1::2]) -> row j of xi is original row perm[j]
    return np.concatenate([np.arange(0, D_ROPE, 2), np.arange(1, D_ROPE, 2)])


def _host_prep(inputs):
    """Build per-core input maps. Returns list of dicts."""
    hidden = np.asarray(inputs["hidden_states"], np.float32)[0]  # [S, HIDDEN]
    q_w = np.asarray(inputs["q_w"], np.float32)
    kv_a_w = np.asarray(inputs["kv_a_w"], np.float32)
    ln_g = np.asarray(inputs["ln_g"], np.float32)
    kv_b_w = np.asarray(inputs["kv_b_w"], np.float32)
    o_w = np.asarray(inputs["o_w"], np.float32)
    cos_c = np.asarray(inputs["cos_cached"], np.float32)
    sin_c = np.asarray(inputs["sin_cached"], np.float32)
    pos = np.asarray(inputs["position_ids"])[0].astype(np.int64)

    perm = _xi_perm()
    cos_p = cos_c[pos]  # [S, 64]
    sin_p = sin_c[pos]
    cosT = np.ascontiguousarray(cos_p.T)  # [64, S]
    sinT = np.ascontiguousarray(sin_p.T)
    sinTs = sinT.copy()
    sinTs[0:32] = -sinTs[0:32]
    cos2 = np.ascontiguousarray(np.concatenate([cosT, cosT], 0).astype(BF16))
    sin2 = np.ascontiguousarray(np.concatenate([sinTs, sinTs], 0).astype(BF16))

    # hN: [NN, NPC, 128, KP*NSL]; hN[n,pc,p,(kk,c)] = hidden[n*NSL+c, (pc*KP+kk)*128+p]
    hT = hidden.T.astype(BF16)  # [HIDDEN, S]
    hN = (hT.reshape(NPC, KP, 128, NN, NSL)
            .transpose(3, 0, 2, 1, 4)
            .reshape(NN, NPC, 128, KP * NSL))
    hN = np.ascontiguousarray(hN)

    # rotate-half permutation: out = P @ x with P = blockdiag(P64, P64),
    # P64[j, (j+32) % 64] = 1  (signs folded into sin2); lhsT = P.T
    P64 = np.zeros((64, 64), np.float32)
    for j in range(64):
        P64[j, (j + 32) % 64] = 1.0
    P128 = np.zeros((128, 128), np.float32)
    P128[:64, :64] = P64
    P128[64:, 64:] = P64
    permT_np = np.ascontiguousarray(P128.T.astype(BF16))

    tri = (np.arange(128)[:, None] <= np.arange(128)[None, :]).astype(np.float32)
    tri01_np = np.ascontiguousarray(tri.astype(BF16))

    kvb_folded = kv_b_w * ln_g[None, :]
    qw_s = q_w * SCALE

    # kvaw extended: 512 latent + rope(perm) duplicated to 128 rows
    kva_rope = kv_a_w[KV_LORA:][perm]  # [64, HIDDEN]
    kvaw_ext = np.concatenate([kv_a_w[:KV_LORA], kva_rope, kva_rope], 0)  # [640, HIDDEN]
    kvawT_p = np.ascontiguousarray(
        kvaw_ext.T.reshape(KT, 128, KVA_COLS).transpose(1, 0, 2)
        .reshape(128, KT * KVA_COLS).astype(BF16))

    in_maps = []
    for c in range(NCORES):
        heads = list(range(c * HC, (c + 1) * HC))
        nope_rows = np.concatenate([np.arange(h * Q_HEAD, h * Q_HEAD + D_NOPE) for h in heads])
        rope_rows = np.concatenate([h * Q_HEAD + D_NOPE + perm for h in heads])
        qw_c = qw_s[np.concatenate([nope_rows, rope_rows])]  # [768, HIDDEN]
        qwT_p = np.ascontiguousarray(
            qw_c.T.reshape(KT, 128, QW_COLS).transpose(1, 0, 2)
            .reshape(128, KT * QW_COLS).astype(BF16))

        knope_rows = np.concatenate([np.arange(h * 256, h * 256 + D_NOPE) for h in heads])
        v_rows = np.concatenate([np.arange(h * 256 + D_NOPE, (h + 1) * 256) for h in heads])
        kbw_c = np.concatenate([kvb_folded[knope_rows], kvb_folded[v_rows]], 0)  # [1024, 512]
        kbw_p = np.ascontiguousarray(
            kbw_c.T.reshape(4, 128, 1024).transpose(1, 0, 2)
            .reshape(128, 4 * 1024).astype(BF16))

        ow_c = o_w[:, c * HC * D_V:(c + 1) * HC * D_V]  # [HIDDEN, 512]
        owT_p = np.ascontiguousarray(
            ow_c.T.reshape(4, 128, HIDDEN).transpose(1, 0, 2)
            .reshape(128, 4 * HIDDEN).astype(BF16))

        # per-core latent slice of hidden: columns [c*LSL, (c+1)*LSL)
        hL_c = (hT[:, c * LSL:(c + 1) * LSL]
                .reshape(2, 16, 128, LSL)
                .transpose(0, 2, 1, 3)
                .reshape(2, 128, 16 * LSL))
        hL_c = np.ascontiguousarray(hL_c)
        csL = np.ascontiguousarray(cos2[:, c * LSL:(c + 1) * LSL])
        snL = np.ascontiguousarray(sin2[:, c * LSL:(c + 1) * LSL])

        m = {"hN": hN, "hL": hL_c, "qwT": qwT_p, "kvawT": kvawT_p, "kbw": kbw_p,
             "owT": owT_p, "cos2": cos2, "sin2": sin2, "csLp": csL, "snLp": snL,
             "permT": permT_np, "tri01": tri01_np}
        in_maps.append(m)
    return in_maps


def _mask_is_causal(mask):
    m = np.asarray(mask, np.float32)[0, 0]
    tri = np.tril(np.ones((S, S), bool))
    return m.shape == (S, S) and np.all(m[tri] == 0.0) and np.all(m[~tri] <= -1e8)


def _numpy_fallback(inputs):
    hs = np.asarray(inputs["hidden_states"], np.float32)
    mask = np.asarray(inputs["attention_mask"], np.float32)
    pos = np.asarray(inputs["position_ids"]).astype(np.int64)
    q_w = np.asarray(inputs["q_w"], np.float32)
    kv_a_w = np.asarray(inputs["kv_a_w"], np.float32)
    ln_g = np.asarray(inputs["ln_g"], np.float32)
    ln_b = np.asarray(inputs["ln_b"], np.float32)
    kv_b_w = np.asarray(inputs["kv_b_w"], np.float32)
    o_w = np.asarray(inputs["o_w"], np.float32)
    cos_c = np.asarray(inputs["cos_cached"], np.float32)
    sin_c = np.asarray(inputs["sin_cached"], np.float32)
    B, Sq, _ = hs.shape
    q = (hs @ q_w.T).reshape(B, Sq, H, Q_HEAD).transpose(0, 2, 1, 3)
    q_nope, q_pe = q[..., :D_NOPE], q[..., D_NOPE:]
    ckv = hs @ kv_a_w.T
    ckv_l, k_pe = ckv[..., :KV_LORA], ckv[..., KV_LORA:]
    k_pe = k_pe[:, None]
    mu = ckv_l.mean(-1, keepdims=True)
    var = ((ckv_l - mu) ** 2).mean(-1, keepdims=True)
    ln = (ckv_l - mu) / np.sqrt(var + LN_EPS) * ln_g + ln_b
    kv = (ln @ kv_b_w.T).reshape(B, Sq, H, D_NOPE + D_V).transpose(0, 2, 1, 3)
    k_nope, v = kv[..., :D_NOPE], kv[..., D_NOPE:]
    cos = cos_c[pos][:, None]
    sin = sin_c[pos][:, None]

    def rope(x):
        xi = np.concatenate([x[..., 0::2], x[..., 1::2]], -1)
        half = xi.shape[-1] // 2
        rot = np.concatenate([-xi[..., half:], xi[..., :half]], -1)
        return xi * cos + rot * sin

    q_pe, k_pe = rope(q_pe), rope(k_pe)
    query = np.concatenate([q_nope, q_pe], -1)
    key = np.concatenate([k_nope, np.broadcast_to(k_pe, (B, H, Sq, D_ROPE))], -1)
    sc = np.einsum("bhqd,bhkd->bhqk", query, key) * SCALE + mask
    sc = sc - sc.max(-1, keepdims=True)
    a = np.exp(sc)
    a /= a.sum(-1, keepdims=True)
    o = np.einsum("bhqk,bhkd->bhqd", a, v)
    o = o.transpose(0, 2, 1, 3).reshape(B, Sq, H * D_V)
    return (o @ o_w.T).astype(np.float32)


def kernel(**inputs):
    if not _mask_is_causal(inputs["attention_mask"]):
        return _numpy_fallback(inputs)
    pos = np.asarray(inputs["position_ids"])[0].astype(np.int64)
    if pos.shape[0] != S or np.asarray(inputs["hidden_states"]).shape != (1, S, HIDDEN):
        return _numpy_fallback(inputs)
    if np.any(np.asarray(inputs["ln_b"], np.float32) != 0.0):
        return _numpy_fallback(inputs)

    from concourse.bass_utils import run_bass_kernel_spmd

    in_maps = _host_prep(inputs)
    if "prog" not in _CACHE:
        _CACHE["prog"] = _build_program()
    nc = _CACHE["prog"]
    res = run_bass_kernel_spmd(nc, in_maps, core_ids=list(range(NCORES)))
    parts = [np.asarray(res.results[i]["out"], np.float32) for i in range(NCORES)]
    total = np.sum(np.stack(parts, 0), 0, dtype=np.float32)
    return total.reshape(1, S, HIDDEN)


# revision 46
# speedup vs baseline: 1.1018x; 1.0492x over previous
"""MockDeepSeekAttention (MLA) fused kernel for 8 TRN2 NeuronCores.

Sharding: tensor-parallel over heads (4 heads/core) for q/kv_b/attention/
o_proj; the shared kv_a latent projection + LN + k-rope is column-sharded
across the 8 cores and all-gathered on-chip (it would otherwise be
replicated work). The host sums the 8 partial o_proj outputs.

Kernel structure (per core):
  L: latent projection for this core's 256-token column slice + LN +
     k-RoPE, packed and AllGather'd via DRAM bounce buffers. The gather
     completes in the background of phase B.
  B: q projections (nope + rope) for this core's 4 heads, N=512 chains,
     fully SBUF-resident.
  C: kv_b projections (k_nope per head, V blocks) from the gathered latent.
  D: causal attention, scores^T layout; softmax sum via ones-matmul;
     mask via sub-sliced exp + memset + [128,128] triangle multiply.
  E: partial o_proj, h-middle loop order so each stationary tile serves
     4 matmuls; bf16 partial output (host sums in f32).
"""

import sys

if "/opt/trn_rl_repo" not in sys.path:
    sys.path.insert(0, "/opt/trn_rl_repo")

import numpy as np
import ml_dtypes

BF16 = ml_dtypes.bfloat16

# Model dims (hardcoded per problem spec)
H = 32
D_NOPE = 128
D_ROPE = 64
D_V = 128
Q_HEAD = D_NOPE + D_ROPE  # 192
KV_LORA = 512
HIDDEN = 4096
S = 2048
NCORES = 8
HC = H // NCORES  # 4 heads per core
SCALE = 1.0 / Q_HEAD**0.5
LN_EPS = 1e-5

KT = HIDDEN // 128   # 32 k-tiles
NSL = 512            # column tile / attention sq tile
NN = S // NSL        # 4
KP = 8               # k-tiles per DMA piece
NPC = KT // KP       # 4 pieces per column tile
LSL = S // NCORES    # 256: per-core latent column slice

QW_COLS = HC * 128 + HC * 64  # 768
KVA_COLS = 640                # 512 latent + 128 duplicated rope

_CACHE = {}


def _build_program():
    import concourse.bass as bass
    import concourse.tile as tile
    from concourse import mybir, bacc
    from contextlib import ExitStack

    f32 = mybir.dt.float32
    bf16 = mybir.dt.bfloat16
    Exp = mybir.ActivationFunctionType.Exp
    Sqrt = mybir.ActivationFunctionType.Sqrt
    Copy = mybir.ActivationFunctionType.Copy

    nc = bacc.Bacc()
    # DRAM params, already in SBUF (partition-major) layout from the host.
    hN = nc.declare_dram_parameter("hN", [NN, NPC, 128, KP * NSL], bf16, isOutput=False)
    hL = nc.declare_dram_parameter("hL", [2, 128, 16 * LSL], bf16, isOutput=False)
    qwT = nc.declare_dram_parameter("qwT", [128, KT * QW_COLS], bf16, isOutput=False)
    kvawT = nc.declare_dram_parameter("kvawT", [128, KT * KVA_COLS], bf16, isOutput=False)
    kbw = nc.declare_dram_parameter("kbw", [128, 4 * 1024], bf16, isOutput=False)
    owT = nc.declare_dram_parameter("owT", [128, HC * HIDDEN], bf16, isOutput=False)
    cos2 = nc.declare_dram_parameter("cos2", [128, S], bf16, isOutput=False)
    sin2 = nc.declare_dram_parameter("sin2", [128, S], bf16, isOutput=False)
    csLp = nc.declare_dram_parameter("csLp", [128, LSL], bf16, isOutput=False)
    snLp = nc.declare_dram_parameter("snLp", [128, LSL], bf16, isOutput=False)
    permT = nc.declare_dram_parameter("permT", [128, 128], bf16, isOutput=False)
    tri01 = nc.declare_dram_parameter("tri01", [128, 128], bf16, isOutput=False)
    out = nc.declare_dram_parameter("out", [S, HIDDEN], bf16, isOutput=True)

    with tile.TileContext(nc) as tc, ExitStack() as ctx:
        p1 = ctx.enter_context(tc.tile_pool(name="p1", bufs=1))
        p2 = ctx.enter_context(tc.tile_pool(name="p2", bufs=2))
        prow = ctx.enter_context(tc.tile_pool(name="prow", bufs=1))
        ph = ctx.enter_context(tc.tile_pool(name="ph", bufs=6))
        pA = ctx.enter_context(tc.tile_pool(name="pA", bufs=6))
        pCh = ctx.enter_context(tc.tile_pool(name="pCh", bufs=4, space="PSUM"))
        pAcc = ctx.enter_context(tc.tile_pool(name="pAcc", bufs=2, space="PSUM"))
        pRow = ctx.enter_context(tc.tile_pool(name="pRow", bufs=1, space="PSUM"))
        pBc = ctx.enter_context(tc.tile_pool(name="pBc", bufs=1, space="PSUM"))
        dram = ctx.enter_context(tc.tile_pool(name="dram", bufs=1, space="DRAM"))

        # ---- constants ----
        ones_col = p1.tile([128, 1], bf16, name="ones_col", tag="ones_col")
        nc.vector.memset(ones_col, 1.0)
        # all-ones rows at partitions 0 and 32 (lhsT for rank-1 broadcasts;
        # partition 32 serves the second head of an attention pair)
        ones_row = p1.tile([33, 128], bf16, name="ones_row", tag="ones_row")
        nc.vector.memset(ones_row, 1.0)
        eps_t = p1.tile([1, 1], f32, name="eps", tag="eps")
        nc.vector.memset(eps_t, LN_EPS)
        perm_sb = p1.tile([128, 128], bf16, name="perm", tag="perm")
        nc.sync.dma_start(out=perm_sb, in_=permT[:, :])
        tri_sb = p1.tile([128, 128], bf16, name="tri", tag="tri")
        nc.sync.dma_start(out=tri_sb, in_=tri01[:, :])

        # ---- persistent activations ----
        qn = [p1.tile([128, S], bf16, name=f"qn{h}", tag=f"qn{h}") for h in range(HC)]
        qpeR = [p1.tile([128, S], bf16, name=f"qpeR{m}", tag=f"qpeR{m}") for m in range(2)]
        # lnT in phases C (gathered); aliased as outT (per head) in phases D/E
        lnT = [p1.tile([128, S], bf16, name=f"lnT{t}", tag=f"lnT{t}") for t in range(4)]
        kpeRd = p1.tile([128, S], bf16, name="kpeRd", tag="kpeRd")

        # ---- big weight buffers (manually phase-shared regions) ----
        # wbuf1: qwT (48KB/part) in B -> owT (32KB) + kn (16KB) after
        wbuf1 = p1.tile([128, KT * QW_COLS], bf16, name="wbuf1", tag="wbuf1")
        # wbuf2: kvawT (40KB/part) in L -> kbw (8KB) + V (16KB) after
        wbuf2 = p1.tile([128, KT * KVA_COLS], bf16, name="wbuf2", tag="wbuf2")
        # Startup DMA plan. The SDMA engines round-robin all queued work at
        # packet granularity, so only per-ring FIFO order gives priority:
        #   scalar ring: hL halves, csL/snL, qw chunks   (phase L + B weights)
        #   sync ring:   kvaw chunks, all four hN n=0 pieces
        # Phases L and B consume hidden-state pieces piece-outer (parallel
        # PSUM chains) so compute starts as soon as the first ~2MB lands.
        def qw_chunk(kc):
            return (wbuf1[:, kc * QW_COLS:(kc + 8) * QW_COLS],
                    qwT[:, kc * QW_COLS:(kc + 8) * QW_COLS])

        lpieces = []
        for pc in range(2):
            hp = ph.tile([128, 16 * LSL], bf16, name="hlp", tag="hp")
            nc.scalar.dma_start(out=hp, in_=hL[pc])
            lpieces.append(hp)
        csL = p1.tile([128, LSL], bf16, name="csL", tag="csL")
        snL = p1.tile([128, LSL], bf16, name="snL", tag="snL")
        nc.scalar.dma_start(out=csL, in_=csLp[:, :])
        nc.scalar.dma_start(out=snL, in_=snLp[:, :])
        for kc in range(0, KT, 8):
            nc.sync.dma_start(out=wbuf2[:, kc * KVA_COLS:(kc + 8) * KVA_COLS],
                              in_=kvawT[:, kc * KVA_COLS:(kc + 8) * KVA_COLS])
        n0_pieces = [None] * NPC

        def n0p(pc, eng):
            hp = ph.tile([128, KP * NSL], bf16, name="hp", tag="hp")
            eng.dma_start(out=hp, in_=hN[0, pc])
            n0_pieces[pc] = hp

        # scalar ring: qw c0, p0, qw c1, qw c2, p2 ; sync ring: p1, qw c3, p3
        o, i_ = qw_chunk(0)
        nc.scalar.dma_start(out=o, in_=i_)
        n0p(0, nc.scalar)
        n0p(1, nc.sync)
        o, i_ = qw_chunk(8)
        nc.scalar.dma_start(out=o, in_=i_)
        o, i_ = qw_chunk(24)
        nc.sync.dma_start(out=o, in_=i_)
        o, i_ = qw_chunk(16)
        nc.scalar.dma_start(out=o, in_=i_)
        n0p(2, nc.scalar)
        n0p(3, nc.sync)

        def qw_sl(k, c0, w):  # qw k-tile slice [128, w]
            return wbuf1[:, k * QW_COLS + c0: k * QW_COLS + c0 + w]

        def kva_sl(k, c0, w):
            return wbuf2[:, k * KVA_COLS + c0: k * KVA_COLS + c0 + w]

        OW_OFF = 0                     # owT: [128, 4*4096] = 32KB (wbuf1)
        KN_OFF = HC * HIDDEN           # kn: 4 heads x [128, 2048] = 16KB (wbuf1)
        KBW_OFF = 0                    # kbw: [128, 4*1024] = 8KB (wbuf2)
        V_OFF = 4 * 1024               # V: [128, 16*512] = 16KB (wbuf2)

        def kn_sl(h, c0, w):
            return wbuf1[:, KN_OFF + h * S + c0: KN_OFF + h * S + c0 + w]

        def ow_sl(h, c0, w):
            return wbuf1[:, OW_OFF + h * HIDDEN + c0: OW_OFF + h * HIDDEN + c0 + w]

        def kbw_sl(k, c0, w):
            return wbuf2[:, KBW_OFF + k * 1024 + c0: KBW_OFF + k * 1024 + c0 + w]

        def v_sl(blk, c0, w):  # blk in 0..15 over sk, cols = 4 heads * 128 dv
            return wbuf2[:, V_OFF + blk * 512 + c0: V_OFF + blk * 512 + c0 + w]

        def outt_sl(h, c0, w):  # aliases lnT[h]
            return lnT[h][:, c0:c0 + w]

        def rope_apply(dsl, raw, cs, sn, w):
            """dsl = raw*cos + (P@raw)*sin (signs folded into sin)."""
            ps_r = pBc.tile([128, NSL], f32, name="ps_rope", tag="bc")
            nc.tensor.matmul(ps_r[:, :w], perm_sb, raw, start=True, stop=True)
            nc.vector.tensor_mul(dsl, raw, cs)
            t1 = p2.tile([128, NSL], bf16, name="rope_t1", tag="rope_t1")
            nc.vector.tensor_mul(t1[:, :w], ps_r[:, :w], sn)
            nc.vector.tensor_add(dsl, dsl, t1[:, :w])

        # ============ Phase L: sharded latent + LN + k-rope + AllGather ============
        in_b = dram.tile([128, 5 * LSL], bf16, name="in_b", tag="in_b")
        out_b = dram.tile([NCORES, 128, 5 * LSL], bf16, name="out_b", tag="out_b",
                          addr_space="Shared")

        with nc.named_scope("phL"):
            lnL = p1.tile([128, 5 * LSL], bf16, name="lnL", tag="lnL")
            # piece-outer: 5 parallel PSUM chains consume each 1MB hL half as
            # soon as it lands (4 banks from pCh + 1 from the idle pAcc)
            ps_l = [pCh.tile([128, NSL], f32, name=f"psL{m}", tag="chain")
                    for m in range(4)]
            ps_l.append(pAcc.tile([128, NSL], f32, name="psL4", tag="acc"))
            for pc in range(2):
                # on the last piece, finish the kpe chain first so its ACT
                # evacuation (feeding the rope matmul) starts early
                morder = (4, 0, 1, 2, 3) if pc == 1 else range(5)
                for m in morder:
                    for kk in range(16):
                        k = pc * 16 + kk
                        nc.tensor.matmul(
                            ps_l[m][:, :LSL], kva_sl(k, m * 128, 128),
                            lpieces[pc][:, kk * LSL:(kk + 1) * LSL],
                            start=(k == 0), stop=(k == KT - 1))
            kpe_raw = p2.tile([128, LSL], bf16, name="kpe_raw", tag="praw")
            nc.scalar.activation(out=kpe_raw, in_=ps_l[4][:, :LSL], func=Copy)
            for m in range(4):
                eng = nc.scalar if m % 2 == 0 else nc.vector
                if m % 2 == 0:
                    nc.scalar.activation(out=lnL[:, m * LSL:(m + 1) * LSL],
                                         in_=ps_l[m][:, :LSL], func=Copy)
                else:
                    nc.vector.tensor_copy(lnL[:, m * LSL:(m + 1) * LSL],
                                          ps_l[m][:, :LSL])
            rope_apply(lnL[:, 4 * LSL:5 * LSL], kpe_raw, csL, snL, LSL)

            # LN stats + apply on the 4 latent ranges of lnL
            r_mu = prow.tile([1, LSL], f32, name="r_mu", tag="r_mu")
            r_e2 = prow.tile([1, LSL], f32, name="r_e2", tag="r_e2")
            ps1 = pRow.tile([1, NSL], f32, name="ps1", tag="row")
            for t in range(4):
                nc.tensor.matmul(ps1[:, :LSL], ones_col, lnL[:, t * LSL:(t + 1) * LSL],
                                 start=(t == 0), stop=(t == 3))
            nc.vector.tensor_scalar_mul(r_mu, ps1[:, :LSL], 1.0 / KV_LORA)
            ps2 = pRow.tile([1, NSL], f32, name="ps2", tag="row")
            for t in range(4):
                x2 = p2.tile([128, LSL], bf16, name="x2", tag="x2")
                nc.vector.tensor_mul(x2, lnL[:, t * LSL:(t + 1) * LSL],
                                     lnL[:, t * LSL:(t + 1) * LSL])
                nc.tensor.matmul(ps2[:, :LSL], ones_col, x2, start=(t == 0), stop=(t == 3))
            nc.vector.tensor_scalar_mul(r_e2, ps2[:, :LSL], 1.0 / KV_LORA)
            r_m2 = prow.tile([1, LSL], f32, name="r_m2", tag="r_m2")
            nc.vector.tensor_mul(r_m2, r_mu, r_mu)
            nc.vector.tensor_sub(r_e2, r_e2, r_m2)
            nc.scalar.activation(out=r_e2, in_=r_e2, func=Sqrt, bias=eps_t)
            nc.vector.reciprocal(r_m2, r_e2)          # rstd
            r_rb = prow.tile([1, LSL], bf16, name="r_rb", tag="r_rb")
            nc.vector.tensor_copy(r_rb, r_m2)
            nc.vector.tensor_mul(r_mu, r_mu, r_m2)    # mu*rstd
            r_bb = prow.tile([1, LSL], bf16, name="r_bb", tag="r_bb")
            nc.vector.tensor_copy(r_bb, r_mu)
            bc1 = pBc.tile([128, NSL], f32, name="bc1", tag="bc")
            nc.tensor.matmul(bc1[:, :LSL], ones_row[0:1, :], r_rb, start=True, stop=True)
            for t in range(4):
                nc.vector.tensor_mul(lnL[:, t * LSL:(t + 1) * LSL],
                                     lnL[:, t * LSL:(t + 1) * LSL], bc1[:, :LSL])
            bc2 = pBc.tile([128, NSL], f32, name="bc2", tag="bc")
            nc.tensor.matmul(bc2[:, :LSL], ones_row[0:1, :], r_bb, start=True, stop=True)
            for t in range(4):
                nc.vector.tensor_sub(lnL[:, t * LSL:(t + 1) * LSL],
                                     lnL[:, t * LSL:(t + 1) * LSL], bc2[:, :LSL])

            # pack -> all-gather (gpsimd queue keeps straight-line ordering)
            nc.gpsimd.dma_start(out=in_b, in_=lnL)
            nc.gpsimd.collective_compute(
                "AllGather", mybir.AluOpType.bypass,
                replica_groups=[list(range(NCORES))],
                ins=[in_b.opt()], outs=[out_b.opt()])
            # NOTE: the unpack DMAs are emitted AFTER phase B (below) — they
            # block on the collective, and a blocked dma_start stalls its
            # issuing engine's whole queue (head-of-line), which would starve
            # phase B's hN loads.

        # =================== Phase B: q projections ===================
        # piece-outer: 6 parallel PSUM chains (4 nope + 2 rope) consume each
        # 2MB hN piece as it lands; pieces stream on alternating rings.
        with nc.named_scope("phB"):
            for n in range(NN):
                sl0 = n * NSL
                if n == 0:
                    pieces = n0_pieces
                else:
                    eng = nc.scalar if n % 2 == 1 else nc.sync
                    pieces = []
                    for pc in range(NPC):
                        hp = ph.tile([128, KP * NSL], bf16, name="hp", tag="hp")
                        eng.dma_start(out=hp, in_=hN[n, pc])
                        pieces.append(hp)
                cs = p2.tile([128, NSL], bf16, name="cs", tag="cs", bufs=1)
                sn = p2.tile([128, NSL], bf16, name="sn", tag="sn", bufs=1)
                nc.scalar.dma_start(out=cs, in_=cos2[:, sl0:sl0 + NSL])
                nc.scalar.dma_start(out=sn, in_=sin2[:, sl0:sl0 + NSL])

                ps_b = [pCh.tile([128, NSL], f32, name=f"psB{m}", tag="chain")
                        for m in range(4)]
                ps_b += [pAcc.tile([128, NSL], f32, name=f"psB{m+4}", tag="acc")
                         for m in range(2)]
                for pc in range(NPC):
                    # last piece: rope chains (m=4,5) stop first so their ACT
                    # evacuations feed the rope matmuls without a PE stall
                    morder = (4, 5, 0, 1, 2, 3) if pc == NPC - 1 else range(6)
                    for m in morder:
                        for kk in range(KP):
                            k = pc * KP + kk
                            nc.tensor.matmul(
                                ps_b[m], qw_sl(k, m * 128, 128),
                                pieces[pc][:, kk * NSL:(kk + 1) * NSL],
                                start=(k == 0), stop=(k == KT - 1))
                q_raws = []
                for m in range(2):
                    q_raw = p2.tile([128, NSL], bf16, name="q_raw", tag="praw")
                    nc.scalar.activation(out=q_raw, in_=ps_b[4 + m], func=Copy)
                    q_raws.append(q_raw)
                for h in range(HC):
                    if h % 2 == 0:
                        nc.scalar.activation(out=qn[h][:, sl0:sl0 + NSL],
                                             in_=ps_b[h], func=Copy)
                    else:
                        nc.vector.tensor_copy(qn[h][:, sl0:sl0 + NSL], ps_b[h])
                for m in range(2):
                    rope_apply(qpeR[m][:, sl0:sl0 + NSL], q_raws[m], cs, sn, NSL)

        # kbw: wbuf2 WAR clears early (end of phase L chains) -> sync queue.
        # owT: wbuf1 WAR clears only when phase B finishes reading qwT, so a
        # blocked dma_start would stall its queue -> park it on gpsimd (idle
        # after the collective).
        nc.sync.dma_start(out=kbw_sl(0, 0, 4 * 1024), in_=kbw[:, :])
        nc.gpsimd.dma_start(out=wbuf1[:, OW_OFF:OW_OFF + HC * HIDDEN], in_=owT[:, :])

        # unpack every core's slice into lnT / kpeRd (after phase B emission
        # so these collective-blocked DMAs sit at the back of the queues)
        with nc.named_scope("phU"):
            for c8 in range(NCORES):
                for t in range(4):
                    eng = nc.sync if (c8 + t) % 2 == 0 else nc.scalar
                    eng.dma_start(out=lnT[t][:, c8 * LSL:(c8 + 1) * LSL],
                                  in_=out_b[c8, :, t * LSL:(t + 1) * LSL])
                eng = nc.sync if c8 % 2 == 0 else nc.scalar
                eng.dma_start(out=kpeRd[:, c8 * LSL:(c8 + 1) * LSL],
                              in_=out_b[c8, :, 4 * LSL:5 * LSL])

        # =================== Phase C: kv_b projections ===================
        with nc.named_scope("phC"):

            for nj in range(NN):
                sl0 = nj * NSL
                for h in range(HC):
                    ps = pCh.tile([128, NSL], f32, name="psKn", tag="chain")
                    for k in range(4):
                        nc.tensor.matmul(ps, kbw_sl(k, h * 128, 128),
                                         lnT[k][:, sl0:sl0 + NSL],
                                         start=(k == 0), stop=(k == 3))
                    nc.scalar.activation(out=kn_sl(h, sl0, NSL), in_=ps, func=Copy)
                for b in range(4):
                    blk = nj * 4 + b
                    ps = pCh.tile([128, NSL], f32, name="psV", tag="chain")
                    for k in range(4):
                        nc.tensor.matmul(ps, lnT[k][:, blk * 128:(blk + 1) * 128],
                                         kbw_sl(k, 512, 512),
                                         start=(k == 0), stop=(k == 3))
                    nc.scalar.activation(out=v_sl(blk, 0, 512), in_=ps, func=Copy)

        # =================== Phase D: attention ===================
        # Heads are processed in pairs (h0 at rows 0-63, h1 at rows 64-127 of
        # the duplicated kpe / stacked q_pe): emitting the two K=64 rope-score
        # matmuls back-to-back lets them run concurrently in disjoint row
        # groups of the PE array.
        def attn_pair(h0, nj, inject=None):
            sl0 = nj * NSL
            nb = 4 * (nj + 1)
            hs = (h0, h0 + 1)
            qpt = qpeR[h0 // 2]
            ps_o = [pAcc.tile([128, NSL], f32, name=f"ps_o{j}", tag="acc")
                    for j in range(2)]
            # both heads' softmax sums share one PSUM bank (partitions 0 / 32,
            # distinct col groups -> the two ones-matmuls can overlap)
            ps_sum2 = pRow.tile([33, NSL], f32, name="ps_sum2", tag="row")
            ps_sum = [ps_sum2[0:1, :], ps_sum2[32:33, :]]

            def sum_v(prev_A, i):
                for j in range(2):  # adjacent: col groups 0/32 can overlap
                    nc.tensor.matmul(ps_sum[j], ones_col, prev_A[j],
                                     start=(i == 0), stop=(i == nb - 1))
                for j in range(2):
                    nc.tensor.matmul(ps_o[j], v_sl(i, hs[j] * 128, 128),
                                     prev_A[j], start=(i == 0), stop=(i == nb - 1))

            prev = None
            for i in range(nb):
                d = i - 4 * nj
                ps_s = [pCh.tile([128, NSL], f32, name=f"ps_s{j}", tag="chain")
                        for j in range(2)]
                for j in range(2):
                    nc.tensor.matmul(ps_s[j], kn_sl(hs[j], i * 128, 128),
                                     qn[hs[j]][:, sl0:sl0 + NSL],
                                     start=True, stop=False)
                for j in range(2):  # adjacent K=64 MMs in disjoint row groups
                    half = 64 * j
                    nc.tensor.matmul(ps_s[j],
                                     kpeRd[half:half + 64, i * 128:(i + 1) * 128],
                                     qpt[half:half + 64, sl0:sl0 + NSL],
                                     start=False, stop=True)
                A_pair = []
                for j in range(2):
                    A_t = pA.tile([128, NSL], bf16, name="A", tag="A")
                    if d < 0:
                        nc.scalar.activation(out=A_t, in_=ps_s[j], func=Exp)
                    else:
                        if d > 0:
                            nc.vector.memset(A_t[:, 0:d * 128], 0.0)
                        nc.scalar.activation(out=A_t[:, d * 128:NSL],
                                             in_=ps_s[j][:, d * 128:NSL], func=Exp)
                        nc.vector.tensor_mul(A_t[:, d * 128:(d + 1) * 128],
                                             A_t[:, d * 128:(d + 1) * 128], tri_sb)
                    A_pair.append(A_t)
                if i == 1 and inject is not None:
                    inject()  # previous pair's normalization (deps long met)
                if prev is not None:
                    sum_v(*prev)  # one block late: aligns the in-order PE
                prev = (A_pair, i)  # queue with actual operand readiness
            sum_v(*prev)
            # normalization closure, emitted one pair late (software
            # pipelining) so its broadcast matmul — which waits on DVE —
            # never heads the in-order PE queue while the next pair computes
            def normalize():
                # one 33-row copy grabs both heads' sums; the two broadcast
                # matmuls go to separate banks (pBc + the freed pRow slot)
                # and disjoint row groups so the PE never queues behind DVE
                r_s = p2.tile([33, NSL], bf16, name="r_s", tag="r_s")
                nc.vector.tensor_copy(r_s[0:1, :], ps_sum2[0:1, :])
                nc.vector.tensor_copy(r_s[32:33, :], ps_sum2[32:33, :])
                bc_r = [pBc.tile([128, NSL], f32, name="bc_r0", tag="bc"),
                        pRow.tile([128, NSL], f32, name="bc_r1", tag="row")]
                for j in range(2):
                    p0 = 32 * j
                    nc.tensor.matmul(bc_r[j], ones_row[p0:p0 + 1, :],
                                     r_s[p0:p0 + 1, :], start=True, stop=True)
                for j in range(2):
                    rec = p2.tile([128, NSL], bf16, name="rec", tag="rec")
                    with nc.allow_low_precision(reason="softmax denom bf16"):
                        nc.vector.reciprocal(rec, bc_r[j])
                    raw_o = p2.tile([128, NSL], bf16, name="raw_o", tag="raw_o")
                    nc.vector.tensor_copy(raw_o, ps_o[j])  # DVE: ACT busy w/ exp
                    nc.vector.tensor_mul(outt_sl(hs[j], sl0, NSL), raw_o, rec)
            return normalize

        with nc.named_scope("phD"):
            pending = None
            for nj in range(NN):
                for h0 in (0, 2):
                    pending = attn_pair(h0, nj, inject=pending)
            pending()

        # =================== Phase E: partial o_proj ===================
        with nc.named_scope("phE"):
            for ms in range(16):
                ostg = ph.tile([128, HIDDEN], bf16, name="ostg", tag="hp")
                for g in range(2):  # ns groups of 4 (uses 4 PSUM banks)
                    pss = [pCh.tile([128, NSL], f32, name="psE", tag="chain")
                           for _ in range(4)]
                    for hh in range(HC):
                        for q in range(4):
                            ns = g * 4 + q
                            nc.tensor.matmul(
                                pss[q], outt_sl(hh, ms * 128, 128),
                                ow_sl(hh, ns * NSL, NSL),
                                start=(hh == 0), stop=(hh == HC - 1))
                    for q in range(4):
                        ns = g * 4 + q
                        dst = ostg[:, ns * NSL:(ns + 1) * NSL]
                        if q % 2 == 0:
                            nc.vector.tensor_copy(dst, pss[q])
                        else:
                            nc.scalar.activation(out=dst, in_=pss[q], func=Copy)
                nc.sync.dma_start(out=out[ms * 128:(ms + 1) * 128, :], in_=ostg)

    nc.finalize()
    return nc


def _xi_perm():
    # xi = concat(x[0::2], x[1::2]) -> row j of xi is original row perm[j]
    return np.concatenate([np.arange(0, D_ROPE, 2), np.arange(1, D_ROPE, 2)])


def _host_prep(inputs):
    """Build per-core input maps. Returns list of dicts."""
    hidden = np.asarray(inputs["hidden_states"], np.float32)[0]  # [S, HIDDEN]
    q_w = np.asarray(inputs["q_w"], np.float32)
    kv_a_w = np.asarray(inputs["kv_a_w"], np.float32)
    ln_g = np.asarray(inputs["ln_g"], np.float32)
    kv_b_w = np.asarray(inputs["kv_b_w"], np.float32)
    o_w = np.asarray(inputs["o_w"], np.float32)
    cos_c = np.asarray(inputs["cos_cached"], np.float32)
    sin_c = np.asarray(inputs["sin_cached"], np.float32)
    pos = np.asarray(inputs["position_ids"])[0].astype(np.int64)

    perm = _xi_perm()
    cos_p = cos_c[pos]  # [S, 64]
    sin_p = sin_c[pos]
    cosT = np.ascontiguousarray(cos_p.T)  # [64, S]
    sinT = np.ascontiguousarray(sin_p.T)
    sinTs = sinT.copy()
    sinTs[0:32] = -sinTs[0:32]
    cos2 = np.ascontiguousarray(np.concatenate([cosT, cosT], 0).astype(BF16))
    sin2 = np.ascontiguousarray(np.concatenate([sinTs, sinTs], 0).astype(BF16))

    # hN: [NN, NPC, 128, KP*NSL]; hN[n,pc,p,(kk,c)] = hidden[n*NSL+c, (pc*KP+kk)*128+p]
    hT = hidden.T.astype(BF16)  # [HIDDEN, S]
    hN = (hT.reshape(NPC, KP, 128, NN, NSL)
            .transpose(3, 0, 2, 1, 4)
            .reshape(NN, NPC, 128, KP * NSL))
    hN = np.ascontiguousarray(hN)

    # rotate-half permutation: out = P @ x with P = blockdiag(P64, P64),
    # P64[j, (j+32) % 64] = 1  (signs folded into sin2); lhsT = P.T
    P64 = np.zeros((64, 64), np.float32)
    for j in range(64):
        P64[j, (j + 32) % 64] = 1.0
    P128 = np.zeros((128, 128), np.float32)
    P128[:64, :64] = P64
    P128[64:, 64:] = P64
    permT_np = np.ascontiguousarray(P128.T.astype(BF16))

    tri = (np.arange(128)[:, None] <= np.arange(128)[None, :]).astype(np.float32)
    tri01_np = np.ascontiguousarray(tri.astype(BF16))

    kvb_folded = kv_b_w * ln_g[None, :]
    qw_s = q_w * SCALE

    # kvaw extended: 512 latent + rope(perm) duplicated to 128 rows
    kva_rope = kv_a_w[KV_LORA:][perm]  # [64, HIDDEN]
    kvaw_ext = np.concatenate([kv_a_w[:KV_LORA], kva_rope, kva_rope], 0)  # [640, HIDDEN]
    kvawT_p = np.ascontiguousarray(
        kvaw_ext.T.reshape(KT, 128, KVA_COLS).transpose(1, 0, 2)
        .reshape(128, KT * KVA_COLS).astype(BF16))

    in_maps = []
    for c in range(NCORES):
        heads = list(range(c * HC, (c + 1) * HC))
        nope_rows = np.concatenate([np.arange(h * Q_HEAD, h * Q_HEAD + D_NOPE) for h in heads])
        rope_rows = np.concatenate([h * Q_HEAD + D_NOPE + perm for h in heads])
        qw_c = qw_s[np.concatenate([nope_rows, rope_rows])]  # [768, HIDDEN]
        qwT_p = np.ascontiguousarray(
            qw_c.T.reshape(KT, 128, QW_COLS).transpose(1, 0, 2)
            .reshape(128, KT * QW_COLS).astype(BF16))

        knope_rows = np.concatenate([np.arange(h * 256, h * 256 + D_NOPE) for h in heads])
        v_rows = np.concatenate([np.arange(h * 256 + D_NOPE, (h + 1) * 256) for h in heads])
        kbw_c = np.concatenate([kvb_folded[knope_rows], kvb_folded[v_rows]], 0)  # [1024, 512]
        kbw_p = np.ascontiguousarray(
            kbw_c.T.reshape(4, 128, 1024).transpose(1, 0, 2)
            .reshape(128, 4 * 1024).astype(BF16))

        ow_c = o_w[:, c * HC * D_V:(c + 1) * HC * D_V]  # [HIDDEN, 512]
        owT_p = np.ascontiguousarray(
            ow_c.T.reshape(4, 128, HIDDEN).transpose(1, 0, 2)
            .reshape(128, 4 * HIDDEN).astype(BF16))

        # per-core latent slice of hidden: columns [c*LSL, (c+1)*LSL)
        hL_c = (hT[:, c * LSL:(c + 1) * LSL]
                .reshape(2, 16, 128, LSL)
                .transpose(0, 2, 1, 3)
                .reshape(2, 128, 16 * LSL))
        hL_c = np.ascontiguousarray(hL_c)
        csL = np.ascontiguousarray(cos2[:, c * LSL:(c + 1) * LSL])
        snL = np.ascontiguousarray(sin2[:, c * LSL:(c + 1) * LSL])

        m = {"hN": hN, "hL": hL_c, "qwT": qwT_p, "kvawT": kvawT_p, "kbw": kbw_p,
             "owT": owT_p, "cos2": cos2, "sin2": sin2, "csLp": csL, "snLp": snL,
             "permT": permT_np, "tri01": tri01_np}
        in_maps.append(m)
    return in_maps


def _mask_is_causal(mask):
    m = np.asarray(mask, np.float32)[0, 0]
    tri = np.tril(np.ones((S, S), bool))
    return m.shape == (S, S) and np.all(m[tri] == 0.0) and np.all(m[~tri] <= -1e8)


def _numpy_fallback(inputs):
    hs = np.asarray(inputs["hidden_states"], np.float32)
    mask = np.asarray(inputs["attention_mask"], np.float32)
    pos = np.asarray(inputs["position_ids"]).astype(np.int64)
    q_w = np.asarray(inputs["q_w"], np.float32)
    kv_a_w = np.asarray(inputs["kv_a_w"], np.float32)
    ln_g = np.asarray(inputs["ln_g"], np.float32)
    ln_b = np.asarray(inputs["ln_b"], np.float32)
    kv_b_w = np.asarray(inputs["kv_b_w"], np.float32)
    o_w = np.asarray(inputs["o_w"], np.float32)
    cos_c = np.asarray(inputs["cos_cached"], np.float32)
    sin_c = np.asarray(inputs["sin_cached"], np.float32)
    B, Sq, _ = hs.shape
    q = (hs @ q_w.T).reshape(B, Sq, H, Q_HEAD).transpose(0, 2, 1, 3)
    q_nope, q_pe = q[..., :D_NOPE], q[..., D_NOPE:]
    ckv = hs @ kv_a_w.T
    ckv_l, k_pe = ckv[..., :KV_LORA], ckv[..., KV_LORA:]
    k_pe = k_pe[:, None]
    mu = ckv_l.mean(-1, keepdims=True)
    var = ((ckv_l - mu) ** 2).mean(-1, keepdims=True)
    ln = (ckv_l - mu) / np.sqrt(var + LN_EPS) * ln_g + ln_b
    kv = (ln @ kv_b_w.T).reshape(B, Sq, H, D_NOPE + D_V).transpose(0, 2, 1, 3)
    k_nope, v = kv[..., :D_NOPE], kv[..., D_NOPE:]
    cos = cos_c[pos][:, None]
    sin = sin_c[pos][:, None]

    def rope(x):
        xi = np.concatenate([x[..., 0::2], x[..., 1::2]], -1)
        half = xi.shape[-1] // 2
        rot = np.concatenate([-xi[..., half:], xi[..., :half]], -1)
        return xi * cos + rot * sin

    q_pe, k_pe = rope(q_pe), rope(k_pe)
    query = np.concatenate([q_nope, q_pe], -1)
    key = np.concatenate([k_nope, np.broadcast_to(k_pe, (B, H, Sq, D_ROPE))], -1)
    sc = np.einsum("bhqd,bhkd->bhqk", query, key) * SCALE + mask
    sc = sc - sc.max(-1, keepdims=True)
    a = np.exp(sc)
    a /= a.sum(-1, keepdims=True)
    o = np.einsum("bhqk,bhkd->bhqd", a, v)
    o = o.transpose(0, 2, 1, 3).reshape(B, Sq, H * D_V)
    return (o @ o_w.T).astype(np.float32)


def kernel(**inputs):
    if not _mask_is_causal(inputs["attention_mask"]):
        return _numpy_fallback(inputs)
    pos = np.asarray(inputs["position_ids"])[0].astype(np.int64)
    if pos.shape[0] != S or np.asarray(inputs["hidden_states"]).shape != (1, S, HIDDEN):
        return _numpy_fallback(inputs)
    if np.any(np.asarray(inputs["ln_b"], np.float32) != 0.0):
        return _numpy_fallback(inputs)

    from concourse.bass_utils import run_bass_kernel_spmd

    in_maps = _host_prep(inputs)
    if "prog" not in _CACHE:
        _CACHE["prog"] = _build_program()
    nc = _CACHE["prog"]
    res = run_bass_kernel_spmd(nc, in_maps, core_ids=list(range(NCORES)))
    parts = [np.asarray(res.results[i]["out"], np.float32) for i in range(NCORES)]
    total = np.sum(np.stack(parts, 0), 0, dtype=np.float32)
    return total.reshape(1, S, HIDDEN)


# revision 49
# speedup vs baseline: 1.1555x; 1.0488x over previous
"""MockDeepSeekAttention (MLA) fused kernel for 8 TRN2 NeuronCores.

Sharding: tensor-parallel over heads (4 heads/core) for q/kv_b/attention/
o_proj; the shared kv_a latent projection + LN + k-rope is column-sharded
across the 8 cores and all-gathered on-chip (it would otherwise be
replicated work). The host sums the 8 partial o_proj outputs.

Kernel structure (per core):
  L: latent projection for this core's 256-token column slice + LN +
     k-RoPE, packed and AllGather'd via DRAM bounce buffers. The gather
     completes in the background of phase B.
  B: q projections (nope + rope) for this core's 4 heads, N=512 chains,
     fully SBUF-resident.
  C: kv_b projections (k_nope per head, V blocks) from the gathered latent.
  D: causal attention, scores^T layout; softmax sum via ones-matmul;
     mask via sub-sliced exp + memset + [128,128] triangle multiply.
  E: partial o_proj, h-middle loop order so each stationary tile serves
     4 matmuls; bf16 partial output (host sums in f32).
"""

import sys

if "/opt/trn_rl_repo" not in sys.path:
    sys.path.insert(0, "/opt/trn_rl_repo")

import numpy as np
import ml_dtypes

BF16 = ml_dtypes.bfloat16

# Model dims (hardcoded per problem spec)
H = 32
D_NOPE = 128
D_ROPE = 64
D_V = 128
Q_HEAD = D_NOPE + D_ROPE  # 192
KV_LORA = 512
HIDDEN = 4096
S = 2048
NCORES = 8
HC = H // NCORES  # 4 heads per core
SCALE = 1.0 / Q_HEAD**0.5
LN_EPS = 1e-5

KT = HIDDEN // 128   # 32 k-tiles
NSL = 512            # column tile / attention sq tile
NN = S // NSL        # 4
KP = 8               # k-tiles per DMA piece
NPC = KT // KP       # 4 pieces per column tile
LSL = S // NCORES    # 256: per-core latent column slice

QW_COLS = HC * 128 + HC * 64  # 768
KVA_COLS = 640                # 512 latent + 128 duplicated rope

_CACHE = {}


def _build_program():
    import concourse.bass as bass
    import concourse.tile as tile
    from concourse import mybir, bacc
    from contextlib import ExitStack

    f32 = mybir.dt.float32
    bf16 = mybir.dt.bfloat16
    Exp = mybir.ActivationFunctionType.Exp
    Sqrt = mybir.ActivationFunctionType.Sqrt
    Copy = mybir.ActivationFunctionType.Copy

    nc = bacc.Bacc()
    # DRAM params, already in SBUF (partition-major) layout from the host.
    hN = nc.declare_dram_parameter("hN", [NN, NPC, 128, KP * NSL], bf16, isOutput=False)
    hL = nc.declare_dram_parameter("hL", [2, 128, 16 * LSL], bf16, isOutput=False)
    qwT = nc.declare_dram_parameter("qwT", [128, KT * QW_COLS], bf16, isOutput=False)
    kvawT = nc.declare_dram_parameter("kvawT", [128, KT * KVA_COLS], bf16, isOutput=False)
    kbw = nc.declare_dram_parameter("kbw", [128, 4 * 1024], bf16, isOutput=False)
    owT = nc.declare_dram_parameter("owT", [128, HC * HIDDEN], bf16, isOutput=False)
    cos2 = nc.declare_dram_parameter("cos2", [128, S], bf16, isOutput=False)
    sin2 = nc.declare_dram_parameter("sin2", [128, S], bf16, isOutput=False)
    csLp = nc.declare_dram_parameter("csLp", [128, LSL], bf16, isOutput=False)
    snLp = nc.declare_dram_parameter("snLp", [128, LSL], bf16, isOutput=False)
    permT = nc.declare_dram_parameter("permT", [128, 128], bf16, isOutput=False)
    tri01 = nc.declare_dram_parameter("tri01", [128, 128], bf16, isOutput=False)
    out = nc.declare_dram_parameter("out", [S, HIDDEN], bf16, isOutput=True)

    with tile.TileContext(nc) as tc, ExitStack() as ctx:
        p1 = ctx.enter_context(tc.tile_pool(name="p1", bufs=1))
        p2 = ctx.enter_context(tc.tile_pool(name="p2", bufs=2))
        prow = ctx.enter_context(tc.tile_pool(name="prow", bufs=1))
        ph = ctx.enter_context(tc.tile_pool(name="ph", bufs=6))
        pA = ctx.enter_context(tc.tile_pool(name="pA", bufs=6))
        pCh = ctx.enter_context(tc.tile_pool(name="pCh", bufs=4, space="PSUM"))
        pAcc = ctx.enter_context(tc.tile_pool(name="pAcc", bufs=2, space="PSUM"))
        pRow = ctx.enter_context(tc.tile_pool(name="pRow", bufs=1, space="PSUM"))
        pBc = ctx.enter_context(tc.tile_pool(name="pBc", bufs=1, space="PSUM"))
        dram = ctx.enter_context(tc.tile_pool(name="dram", bufs=1, space="DRAM"))

        # ---- constants ----
        ones_col = p1.tile([128, 1], bf16, name="ones_col", tag="ones_col")
        nc.vector.memset(ones_col, 1.0)
        # all-ones rows at partitions 0 and 32 (lhsT for rank-1 broadcasts;
        # partition 32 serves the second head of an attention pair)
        ones_row = p1.tile([33, 128], bf16, name="ones_row", tag="ones_row")
        nc.vector.memset(ones_row, 1.0)
        eps_t = p1.tile([1, 1], f32, name="eps", tag="eps")
        nc.vector.memset(eps_t, LN_EPS)
        perm_sb = p1.tile([128, 128], bf16, name="perm", tag="perm")
        nc.sync.dma_start(out=perm_sb, in_=permT[:, :])
        tri_sb = p1.tile([128, 128], bf16, name="tri", tag="tri")
        nc.sync.dma_start(out=tri_sb, in_=tri01[:, :])

        # ---- persistent activations ----
        qn = [p1.tile([128, S], bf16, name=f"qn{h}", tag=f"qn{h}") for h in range(HC)]
        qpeR = [p1.tile([128, S], bf16, name=f"qpeR{m}", tag=f"qpeR{m}") for m in range(2)]
        # lnT in phases C (gathered); aliased as outT (per head) in phases D/E
        lnT = [p1.tile([128, S], bf16, name=f"lnT{t}", tag=f"lnT{t}") for t in range(4)]
        kpeRd = p1.tile([128, S], bf16, name="kpeRd", tag="kpeRd")

        # ---- big weight buffers (manually phase-shared regions) ----
        # wbuf1: qwT (48KB/part) in B -> owT (32KB) + kn (16KB) after
        wbuf1 = p1.tile([128, KT * QW_COLS], bf16, name="wbuf1", tag="wbuf1")
        # wbuf2: kvawT (40KB/part) in L -> kbw (8KB) + V (16KB) after
        wbuf2 = p1.tile([128, KT * KVA_COLS], bf16, name="wbuf2", tag="wbuf2")
        # Startup DMA plan. The SDMA engines round-robin all queued work at
        # packet granularity, so only per-ring FIFO order gives priority:
        #   scalar ring: hL halves, csL/snL, qw chunks   (phase L + B weights)
        #   sync ring:   kvaw chunks, all four hN n=0 pieces
        # Phases L and B consume hidden-state pieces piece-outer (parallel
        # PSUM chains) so compute starts as soon as the first ~2MB lands.
        def qw_chunk(kc):
            return (wbuf1[:, kc * QW_COLS:(kc + 8) * QW_COLS],
                    qwT[:, kc * QW_COLS:(kc + 8) * QW_COLS])

        lpieces = []
        for pc in range(2):
            hp = ph.tile([128, 16 * LSL], bf16, name="hlp", tag="hp")
            nc.scalar.dma_start(out=hp, in_=hL[pc])
            lpieces.append(hp)
        csL = p1.tile([128, LSL], bf16, name="csL", tag="csL")
        snL = p1.tile([128, LSL], bf16, name="snL", tag="snL")
        nc.scalar.dma_start(out=csL, in_=csLp[:, :])
        nc.scalar.dma_start(out=snL, in_=snLp[:, :])
        for kc in range(0, KT, 8):
            nc.sync.dma_start(out=wbuf2[:, kc * KVA_COLS:(kc + 8) * KVA_COLS],
                              in_=kvawT[:, kc * KVA_COLS:(kc + 8) * KVA_COLS])
        n0_pieces = [None] * NPC

        def n0p(pc, eng):
            hp = ph.tile([128, KP * NSL], bf16, name="hp", tag="hp")
            eng.dma_start(out=hp, in_=hN[0, pc])
            n0_pieces[pc] = hp

        # scalar ring: qw c0, p0, qw c1, qw c2, p2 ; sync ring: p1, qw c3, p3
        o, i_ = qw_chunk(0)
        nc.scalar.dma_start(out=o, in_=i_)
        n0p(0, nc.scalar)
        n0p(1, nc.sync)
        o, i_ = qw_chunk(8)
        nc.scalar.dma_start(out=o, in_=i_)
        o, i_ = qw_chunk(24)
        nc.sync.dma_start(out=o, in_=i_)
        o, i_ = qw_chunk(16)
        nc.scalar.dma_start(out=o, in_=i_)
        n0p(2, nc.scalar)
        n0p(3, nc.sync)

        def qw_sl(k, c0, w):  # qw k-tile slice [128, w]
            return wbuf1[:, k * QW_COLS + c0: k * QW_COLS + c0 + w]

        def kva_sl(k, c0, w):
            return wbuf2[:, k * KVA_COLS + c0: k * KVA_COLS + c0 + w]

        OW_OFF = 0                     # owT: [128, 4*4096] = 32KB (wbuf1)
        KN_OFF = HC * HIDDEN           # kn: 4 heads x [128, 2048] = 16KB (wbuf1)
        KBW_OFF = 0                    # kbw: [128, 4*1024] = 8KB (wbuf2)
        V_OFF = 4 * 1024               # V: [128, 16*512] = 16KB (wbuf2)

        def kn_sl(h, c0, w):
            return wbuf1[:, KN_OFF + h * S + c0: KN_OFF + h * S + c0 + w]

        def ow_sl(h, c0, w):
            return wbuf1[:, OW_OFF + h * HIDDEN + c0: OW_OFF + h * HIDDEN + c0 + w]

        def kbw_sl(k, c0, w):
            return wbuf2[:, KBW_OFF + k * 1024 + c0: KBW_OFF + k * 1024 + c0 + w]

        def v_sl(blk, c0, w):  # blk in 0..15 over sk, cols = 4 heads * 128 dv
            return wbuf2[:, V_OFF + blk * 512 + c0: V_OFF + blk * 512 + c0 + w]

        def outt_sl(h, c0, w):  # aliases lnT[h]
            return lnT[h][:, c0:c0 + w]

        def rope_apply(dsl, raw, cs, sn, w):
            """dsl = raw*cos + (P@raw)*sin (signs folded into sin)."""
            ps_r = pBc.tile([128, NSL], f32, name="ps_rope", tag="bc")
            nc.tensor.matmul(ps_r[:, :w], perm_sb, raw, start=True, stop=True)
            nc.vector.tensor_mul(dsl, raw, cs)
            t1 = p2.tile([128, NSL], bf16, name="rope_t1", tag="rope_t1")
            nc.vector.tensor_mul(t1[:, :w], ps_r[:, :w], sn)
            nc.vector.tensor_add(dsl, dsl, t1[:, :w])

        # ============ Phase L: sharded latent + LN + k-rope + AllGather ============
        in_b = dram.tile([128, 5 * LSL], bf16, name="in_b", tag="in_b")
        out_b = dram.tile([NCORES, 128, 5 * LSL], bf16, name="out_b", tag="out_b",
                          addr_space="Shared")

        with nc.named_scope("phL"):
            lnL = p1.tile([128, 5 * LSL], bf16, name="lnL", tag="lnL")
            # piece-outer: 5 parallel PSUM chains consume each 1MB hL half as
            # soon as it lands (4 banks from pCh + 1 from the idle pAcc)
            ps_l = [pCh.tile([128, NSL], f32, name=f"psL{m}", tag="chain")
                    for m in range(4)]
            ps_l.append(pAcc.tile([128, NSL], f32, name="psL4", tag="acc"))
            for pc in range(2):
                # on the last piece, finish the kpe chain first so its ACT
                # evacuation (feeding the rope matmul) starts early
                morder = (4, 0, 1, 2, 3) if pc == 1 else range(5)
                for m in morder:
                    for kk in range(16):
                        k = pc * 16 + kk
                        nc.tensor.matmul(
                            ps_l[m][:, :LSL], kva_sl(k, m * 128, 128),
                            lpieces[pc][:, kk * LSL:(kk + 1) * LSL],
                            start=(k == 0), stop=(k == KT - 1))
            kpe_raw = p2.tile([128, LSL], bf16, name="kpe_raw", tag="praw")
            nc.scalar.activation(out=kpe_raw, in_=ps_l[4][:, :LSL], func=Copy)
            for m in range(4):
                eng = nc.scalar if m % 2 == 0 else nc.vector
                if m % 2 == 0:
                    nc.scalar.activation(out=lnL[:, m * LSL:(m + 1) * LSL],
                                         in_=ps_l[m][:, :LSL], func=Copy)
                else:
                    nc.vector.tensor_copy(lnL[:, m * LSL:(m + 1) * LSL],
                                          ps_l[m][:, :LSL])
            rope_apply(lnL[:, 4 * LSL:5 * LSL], kpe_raw, csL, snL, LSL)

            # LN stats + apply on the 4 latent ranges of lnL
            r_mu = prow.tile([1, LSL], f32, name="r_mu", tag="r_mu")
            r_e2 = prow.tile([1, LSL], f32, name="r_e2", tag="r_e2")
            ps1 = pRow.tile([1, NSL], f32, name="ps1", tag="row")
            for t in range(4):
                nc.tensor.matmul(ps1[:, :LSL], ones_col, lnL[:, t * LSL:(t + 1) * LSL],
                                 start=(t == 0), stop=(t == 3))
            nc.vector.tensor_scalar_mul(r_mu, ps1[:, :LSL], 1.0 / KV_LORA)
            ps2 = pRow.tile([1, NSL], f32, name="ps2", tag="row")
            for t in range(4):
                x2 = p2.tile([128, LSL], bf16, name="x2", tag="x2")
                nc.vector.tensor_mul(x2, lnL[:, t * LSL:(t + 1) * LSL],
                                     lnL[:, t * LSL:(t + 1) * LSL])
                nc.tensor.matmul(ps2[:, :LSL], ones_col, x2, start=(t == 0), stop=(t == 3))
            nc.vector.tensor_scalar_mul(r_e2, ps2[:, :LSL], 1.0 / KV_LORA)
            r_m2 = prow.tile([1, LSL], f32, name="r_m2", tag="r_m2")
            nc.vector.tensor_mul(r_m2, r_mu, r_mu)
            nc.vector.tensor_sub(r_e2, r_e2, r_m2)
            nc.scalar.activation(out=r_e2, in_=r_e2, func=Sqrt, bias=eps_t)
            nc.vector.reciprocal(r_m2, r_e2)          # rstd
            r_rb = prow.tile([1, LSL], bf16, name="r_rb", tag="r_rb")
            nc.vector.tensor_copy(r_rb, r_m2)
            nc.vector.tensor_mul(r_mu, r_mu, r_m2)    # mu*rstd
            r_bb = prow.tile([1, LSL], bf16, name="r_bb", tag="r_bb")
            nc.vector.tensor_copy(r_bb, r_mu)
            bc1 = pBc.tile([128, NSL], f32, name="bc1", tag="bc")
            nc.tensor.matmul(bc1[:, :LSL], ones_row[0:1, :], r_rb, start=True, stop=True)
            for t in range(4):
                nc.vector.tensor_mul(lnL[:, t * LSL:(t + 1) * LSL],
                                     lnL[:, t * LSL:(t + 1) * LSL], bc1[:, :LSL])
            bc2 = pBc.tile([128, NSL], f32, name="bc2", tag="bc")
            nc.tensor.matmul(bc2[:, :LSL], ones_row[0:1, :], r_bb, start=True, stop=True)
            for t in range(4):
                nc.vector.tensor_sub(lnL[:, t * LSL:(t + 1) * LSL],
                                     lnL[:, t * LSL:(t + 1) * LSL], bc2[:, :LSL])

            # pack -> all-gather (gpsimd queue keeps straight-line ordering)
            nc.gpsimd.dma_start(out=in_b, in_=lnL)
            nc.gpsimd.collective_compute(
                "AllGather", mybir.AluOpType.bypass,
                replica_groups=[list(range(NCORES))],
                ins=[in_b.opt()], outs=[out_b.opt()])
            # NOTE: the unpack DMAs are emitted AFTER phase B (below) — they
            # block on the collective, and a blocked dma_start stalls its
            # issuing engine's whole queue (head-of-line), which would starve
            # phase B's hN loads.

        # =================== Phase B: q projections ===================
        # piece-outer: 6 parallel PSUM chains (4 nope + 2 rope) consume each
        # 2MB hN piece as it lands; pieces stream on alternating rings.
        with nc.named_scope("phB"):
            for n in range(NN):
                sl0 = n * NSL
                if n == 0:
                    pieces = n0_pieces
                else:
                    eng = nc.scalar if n % 2 == 1 else nc.sync
                    pieces = []
                    for pc in range(NPC):
                        hp = ph.tile([128, KP * NSL], bf16, name="hp", tag="hp")
                        eng.dma_start(out=hp, in_=hN[n, pc])
                        pieces.append(hp)
                cs = p2.tile([128, NSL], bf16, name="cs", tag="cs", bufs=1)
                sn = p2.tile([128, NSL], bf16, name="sn", tag="sn", bufs=1)
                nc.scalar.dma_start(out=cs, in_=cos2[:, sl0:sl0 + NSL])
                nc.scalar.dma_start(out=sn, in_=sin2[:, sl0:sl0 + NSL])

                ps_b = [pCh.tile([128, NSL], f32, name=f"psB{m}", tag="chain")
                        for m in range(4)]
                ps_b += [pAcc.tile([128, NSL], f32, name=f"psB{m+4}", tag="acc")
                         for m in range(2)]
                for pc in range(NPC):
                    # last piece: rope chains (m=4,5) stop first so their ACT
                    # evacuations feed the rope matmuls without a PE stall
                    morder = (4, 5, 0, 1, 2, 3) if pc == NPC - 1 else range(6)
                    for m in morder:
                        for kk in range(KP):
                            k = pc * KP + kk
                            nc.tensor.matmul(
                                ps_b[m], qw_sl(k, m * 128, 128),
                                pieces[pc][:, kk * NSL:(kk + 1) * NSL],
                                start=(k == 0), stop=(k == KT - 1))
                q_raws = []
                for m in range(2):
                    q_raw = p2.tile([128, NSL], bf16, name="q_raw", tag="praw")
                    nc.scalar.activation(out=q_raw, in_=ps_b[4 + m], func=Copy)
                    q_raws.append(q_raw)
                for h in range(HC):
                    if h % 2 == 0:
                        nc.scalar.activation(out=qn[h][:, sl0:sl0 + NSL],
                                             in_=ps_b[h], func=Copy)
                    else:
                        nc.vector.tensor_copy(qn[h][:, sl0:sl0 + NSL], ps_b[h])
                for m in range(2):
                    rope_apply(qpeR[m][:, sl0:sl0 + NSL], q_raws[m], cs, sn, NSL)

        # kbw: wbuf2 WAR clears early (end of phase L chains) -> sync queue.
        # owT: wbuf1 WAR clears only when phase B finishes reading qwT, so a
        # blocked dma_start would stall its queue -> park it on gpsimd (idle
        # after the collective).
        nc.sync.dma_start(out=kbw_sl(0, 0, 4 * 1024), in_=kbw[:, :])
        nc.gpsimd.dma_start(out=wbuf1[:, OW_OFF:OW_OFF + HC * HIDDEN], in_=owT[:, :])

        # unpack every core's slice into lnT / kpeRd (after phase B emission
        # so these collective-blocked DMAs sit at the back of the queues)
        with nc.named_scope("phU"):
            for c8 in range(NCORES):
                for t in range(4):
                    eng = nc.sync if (c8 + t) % 2 == 0 else nc.scalar
                    eng.dma_start(out=lnT[t][:, c8 * LSL:(c8 + 1) * LSL],
                                  in_=out_b[c8, :, t * LSL:(t + 1) * LSL])
                eng = nc.sync if c8 % 2 == 0 else nc.scalar
                eng.dma_start(out=kpeRd[:, c8 * LSL:(c8 + 1) * LSL],
                              in_=out_b[c8, :, 4 * LSL:5 * LSL])

        # =================== Phase C: kv_b projections ===================
        with nc.named_scope("phC"):

            for nj in range(NN):
                sl0 = nj * NSL
                for h in range(HC):
                    ps = pCh.tile([128, NSL], f32, name="psKn", tag="chain")
                    for k in range(4):
                        nc.tensor.matmul(ps, kbw_sl(k, h * 128, 128),
                                         lnT[k][:, sl0:sl0 + NSL],
                                         start=(k == 0), stop=(k == 3))
                    nc.scalar.activation(out=kn_sl(h, sl0, NSL), in_=ps, func=Copy)
                for b in range(4):
                    blk = nj * 4 + b
                    ps = pCh.tile([128, NSL], f32, name="psV", tag="chain")
                    for k in range(4):
                        nc.tensor.matmul(ps, lnT[k][:, blk * 128:(blk + 1) * 128],
                                         kbw_sl(k, 512, 512),
                                         start=(k == 0), stop=(k == 3))
                    nc.scalar.activation(out=v_sl(blk, 0, 512), in_=ps, func=Copy)

        # =================== Phase D: attention ===================
        # Heads are processed in pairs (h0 at rows 0-63, h1 at rows 64-127 of
        # the duplicated kpe / stacked q_pe): emitting the two K=64 rope-score
        # matmuls back-to-back lets them run concurrently in disjoint row
        # groups of the PE array.
        def attn_pair(h0, nj, inject=None):
            sl0 = nj * NSL
            nb = 4 * (nj + 1)
            hs = (h0, h0 + 1)
            qpt = qpeR[h0 // 2]
            ps_o = [pAcc.tile([128, NSL], f32, name=f"ps_o{j}", tag="acc")
                    for j in range(2)]
            # both heads' softmax sums share one PSUM bank (partitions 0 / 32,
            # distinct col groups -> the two ones-matmuls can overlap)
            ps_sum2 = pRow.tile([33, NSL], f32, name="ps_sum2", tag="row")
            ps_sum = [ps_sum2[0:1, :], ps_sum2[32:33, :]]

            def sum_v(prev_A, i):
                for j in range(2):  # adjacent: col groups 0/32 can overlap
                    nc.tensor.matmul(ps_sum[j], ones_col, prev_A[j],
                                     start=(i == 0), stop=(i == nb - 1))
                for j in range(2):
                    nc.tensor.matmul(ps_o[j], v_sl(i, hs[j] * 128, 128),
                                     prev_A[j], start=(i == 0), stop=(i == nb - 1))

            prev = None
            for i in range(nb):
                d = i - 4 * nj
                ps_s = [pCh.tile([128, NSL], f32, name=f"ps_s{j}", tag="chain")
                        for j in range(2)]
                for j in range(2):
                    nc.tensor.matmul(ps_s[j], kn_sl(hs[j], i * 128, 128),
                                     qn[hs[j]][:, sl0:sl0 + NSL],
                                     start=True, stop=False)
                for j in range(2):  # adjacent K=64 MMs in disjoint row groups
                    half = 64 * j
                    nc.tensor.matmul(ps_s[j],
                                     kpeRd[half:half + 64, i * 128:(i + 1) * 128],
                                     qpt[half:half + 64, sl0:sl0 + NSL],
                                     start=False, stop=True)
                A_pair = []
                for j in range(2):
                    A_t = pA.tile([128, NSL], bf16, name="A", tag="A")
                    if d < 0:
                        nc.scalar.activation(out=A_t, in_=ps_s[j], func=Exp)
                    else:
                        if d > 0:
                            nc.vector.memset(A_t[:, 0:d * 128], 0.0)
                        nc.scalar.activation(out=A_t[:, d * 128:NSL],
                                             in_=ps_s[j][:, d * 128:NSL], func=Exp)
                        nc.vector.tensor_mul(A_t[:, d * 128:(d + 1) * 128],
                                             A_t[:, d * 128:(d + 1) * 128], tri_sb)
                    A_pair.append(A_t)
                if i == 1 and inject is not None:
                    inject()  # previous pair's normalization (deps long met)
                if prev is not None:
                    sum_v(*prev)  # one block late: aligns the in-order PE
                prev = (A_pair, i)  # queue with actual operand readiness
            sum_v(*prev)
            # normalization closure, emitted one pair late (software
            # pipelining) so its broadcast matmul — which waits on DVE —
            # never heads the in-order PE queue while the next pair computes
            def normalize():
                # release ps_sum2 (pRow) fast: row copies split ACT/DVE in
                # parallel; bc_r1 (which reuses the pRow bank) and its
                # reciprocal run FIRST so the bank frees before the next
                # pair's first sum-matmul reaches the in-order PE queue.
                r_s = p2.tile([33, NSL], bf16, name="r_s", tag="r_s")
                nc.vector.tensor_copy(r_s[32:33, :], ps_sum2[32:33, :])
                nc.scalar.activation(out=r_s[0:1, :], in_=ps_sum2[0:1, :], func=Copy)
                bc_r1 = pRow.tile([128, NSL], f32, name="bc_r1", tag="row")
                nc.tensor.matmul(bc_r1, ones_row[32:33, :], r_s[32:33, :],
                                 start=True, stop=True)
                bc_r0 = pBc.tile([128, NSL], f32, name="bc_r0", tag="bc")
                nc.tensor.matmul(bc_r0, ones_row[0:1, :], r_s[0:1, :],
                                 start=True, stop=True)
                bc_r = [bc_r0, bc_r1]
                for j in (1, 0):
                    rec = p2.tile([128, NSL], bf16, name="rec", tag="rec")
                    with nc.allow_low_precision(reason="softmax denom bf16"):
                        nc.vector.reciprocal(rec, bc_r[j])
                    raw_o = p2.tile([128, NSL], bf16, name="raw_o", tag="raw_o")
                    nc.vector.tensor_copy(raw_o, ps_o[j])  # DVE: ACT busy w/ exp
                    nc.vector.tensor_mul(outt_sl(hs[j], sl0, NSL), raw_o, rec)
            return normalize

        with nc.named_scope("phD"):
            pending = None
            for nj in range(NN):
                for h0 in (0, 2):
                    pending = attn_pair(h0, nj, inject=pending)
            pending()

        # =================== Phase E: partial o_proj ===================
        # two ns-groups of 4 chains in flight: even groups use pCh's 4 banks,
        # odd groups the 4 banks of pAcc/pRow/pBc (idle now); evacuations are
        # emitted one group late so chain matmuls never queue behind them.
        with nc.named_scope("phE"):
            def e_group(ms, g):
                if g % 2 == 0:
                    pss = [pCh.tile([128, NSL], f32, name="psE", tag="chain")
                           for _ in range(4)]
                else:
                    pss = [pAcc.tile([128, NSL], f32, name="psEa", tag="acc"),
                           pAcc.tile([128, NSL], f32, name="psEb", tag="acc"),
                           pRow.tile([128, NSL], f32, name="psEc", tag="row"),
                           pBc.tile([128, NSL], f32, name="psEd", tag="bc")]
                for hh in range(HC):
                    for q in range(4):
                        ns = g * 4 + q
                        nc.tensor.matmul(
                            pss[q], outt_sl(hh, ms * 128, 128),
                            ow_sl(hh, ns * NSL, NSL),
                            start=(hh == 0), stop=(hh == HC - 1))

                def evac(ostg_dst):
                    for q in range(4):
                        ns = g * 4 + q
                        dst = ostg_dst[:, ns * NSL:(ns + 1) * NSL]
                        if q % 2 == 0:
                            nc.vector.tensor_copy(dst, pss[q])
                        else:
                            nc.scalar.activation(out=dst, in_=pss[q], func=Copy)
                return evac

            for ms in range(16):
                ostg = ph.tile([128, HIDDEN], bf16, name="ostg", tag="hp")
                ev0 = e_group(ms, 0)
                ev1 = e_group(ms, 1)
                ev0(ostg)
                ev1(ostg)
                nc.sync.dma_start(out=out[ms * 128:(ms + 1) * 128, :], in_=ostg)

    nc.finalize()
    return nc


def _xi_perm():
    # xi = concat(x[0::2], x[1::2]) -> row j of xi is original row perm[j]
    return np.concatenate([np.arange(0, D_ROPE, 2), np.arange(1, D_ROPE, 2)])


def _host_prep(inputs):
    """Build per-core input maps. Returns list of dicts."""
    hidden = np.asarray(inputs["hidden_states"], np.float32)[0]  # [S, HIDDEN]
    q_w = np.asarray(inputs["q_w"], np.float32)
    kv_a_w = np.asarray(inputs["kv_a_w"], np.float32)
    ln_g = np.asarray(inputs["ln_g"], np.float32)
    kv_b_w = np.asarray(inputs["kv_b_w"], np.float32)
    o_w = np.asarray(inputs["o_w"], np.float32)
    cos_c = np.asarray(inputs["cos_cached"], np.float32)
    sin_c = np.asarray(inputs["sin_cached"], np.float32)
    pos = np.asarray(inputs["position_ids"])[0].astype(np.int64)

    perm = _xi_perm()
    cos_p = cos_c[pos]  # [S, 64]
    sin_p = sin_c[pos]
    cosT = np.ascontiguousarray(cos_p.T)  # [64, S]
    sinT = np.ascontiguousarray(sin_p.T)
    sinTs = sinT.copy()
    sinTs[0:32] = -sinTs[0:32]
    cos2 = np.ascontiguousarray(np.concatenate([cosT, cosT], 0).astype(BF16))
    sin2 = np.ascontiguousarray(np.concatenate([sinTs, sinTs], 0).astype(BF16))

    # hN: [NN, NPC, 128, KP*NSL]; hN[n,pc,p,(kk,c)] = hidden[n*NSL+c, (pc*KP+kk)*128+p]
    hT = hidden.T.astype(BF16)  # [HIDDEN, S]
    hN = (hT.reshape(NPC, KP, 128, NN, NSL)
            .transpose(3, 0, 2, 1, 4)
            .reshape(NN, NPC, 128, KP * NSL))
    hN = np.ascontiguousarray(hN)

    # rotate-half permutation: out = P @ x with P = blockdiag(P64, P64),
    # P64[j, (j+32) % 64] = 1  (signs folded into sin2); lhsT = P.T
    P64 = np.zeros((64, 64), np.float32)
    for j in range(64):
        P64[j, (j + 32) % 64] = 1.0
    P128 = np.zeros((128, 128), np.float32)
    P128[:64, :64] = P64
    P128[64:, 64:] = P64
    permT_np = np.ascontiguousarray(P128.T.astype(BF16))

    tri = (np.arange(128)[:, None] <= np.arange(128)[None, :]).astype(np.float32)
    tri01_np = np.ascontiguousarray(tri.astype(BF16))

    kvb_folded = kv_b_w * ln_g[None, :]
    qw_s = q_w * SCALE

    # kvaw extended: 512 latent + rope(perm) duplicated to 128 rows
    kva_rope = kv_a_w[KV_LORA:][perm]  # [64, HIDDEN]
    kvaw_ext = np.concatenate([kv_a_w[:KV_LORA], kva_rope, kva_rope], 0)  # [640, HIDDEN]
    kvawT_p = np.ascontiguousarray(
        kvaw_ext.T.reshape(KT, 128, KVA_COLS).transpose(1, 0, 2)
        .reshape(128, KT * KVA_COLS).astype(BF16))

    in_maps = []
    for c in range(NCORES):
        heads = list(range(c * HC, (c + 1) * HC))
        nope_rows = np.concatenate([np.arange(h * Q_HEAD, h * Q_HEAD + D_NOPE) for h in heads])
        rope_rows = np.concatenate([h * Q_HEAD + D_NOPE + perm for h in heads])
        qw_c = qw_s[np.concatenate([nope_rows, rope_rows])]  # [768, HIDDEN]
        qwT_p = np.ascontiguousarray(
            qw_c.T.reshape(KT, 128, QW_COLS).transpose(1, 0, 2)
            .reshape(128, KT * QW_COLS).astype(BF16))

        knope_rows = np.concatenate([np.arange(h * 256, h * 256 + D_NOPE) for h in heads])
        v_rows = np.concatenate([np.arange(h * 256 + D_NOPE, (h + 1) * 256) for h in heads])
        kbw_c = np.concatenate([kvb_folded[knope_rows], kvb_folded[v_rows]], 0)  # [1024, 512]
        kbw_p = np.ascontiguousarray(
            kbw_c.T.reshape(4, 128, 1024).transpose(1, 0, 2)
            .reshape(128, 4 * 1024).astype(BF16))

        ow_c = o_w[:, c * HC * D_V:(c + 1) * HC * D_V]  # [HIDDEN, 512]
        owT_p = np.ascontiguousarray(
            ow_c.T.reshape(4, 128, HIDDEN).transpose(1, 0, 2)
            .reshape(128, 4 * HIDDEN).astype(BF16))

        # per-core latent slice of hidden: columns [c*LSL, (c+1)*LSL)
        hL_c = (hT[:, c * LSL:(c + 1) * LSL]
                .reshape(2, 16, 128, LSL)
                .transpose(0, 2, 1, 3)
                .reshape(2, 128, 16 * LSL))
        hL_c = np.ascontiguousarray(hL_c)
        csL = np.ascontiguousarray(cos2[:, c * LSL:(c + 1) * LSL])
        snL = np.ascontiguousarray(sin2[:, c * LSL:(c + 1) * LSL])

        m = {"hN": hN, "hL": hL_c, "qwT": qwT_p, "kvawT": kvawT_p, "kbw": kbw_p,
             "owT": owT_p, "cos2": cos2, "sin2": sin2, "csLp": csL, "snLp": snL,
             "permT": permT_np, "tri01": tri01_np}
        in_maps.append(m)
    return in_maps


def _mask_is_causal(mask):
    m = np.asarray(mask, np.float32)[0, 0]
    tri = np.tril(np.ones((S, S), bool))
    return m.shape == (S, S) and np.all(m[tri] == 0.0) and np.all(m[~tri] <= -1e8)


def _numpy_fallback(inputs):
    hs = np.asarray(inputs["hidden_states"], np.float32)
    mask = np.asarray(inputs["attention_mask"], np.float32)
    pos = np.asarray(inputs["position_ids"]).astype(np.int64)
    q_w = np.asarray(inputs["q_w"], np.float32)
    kv_a_w = np.asarray(inputs["kv_a_w"], np.float32)
    ln_g = np.asarray(inputs["ln_g"], np.float32)
    ln_b = np.asarray(inputs["ln_b"], np.float32)
    kv_b_w = np.asarray(inputs["kv_b_w"], np.float32)
    o_w = np.asarray(inputs["o_w"], np.float32)
    cos_c = np.asarray(inputs["cos_cached"], np.float32)
    sin_c = np.asarray(inputs["sin_cached"], np.float32)
    B, Sq, _ = hs.shape
    q = (hs @ q_w.T).reshape(B, Sq, H, Q_HEAD).transpose(0, 2, 1, 3)
    q_nope, q_pe = q[..., :D_NOPE], q[..., D_NOPE:]
    ckv = hs @ kv_a_w.T
    ckv_l, k_pe = ckv[..., :KV_LORA], ckv[..., KV_LORA:]
    k_pe = k_pe[:, None]
    mu = ckv_l.mean(-1, keepdims=True)
    var = ((ckv_l - mu) ** 2).mean(-1, keepdims=True)
    ln = (ckv_l - mu) / np.sqrt(var + LN_EPS) * ln_g + ln_b
    kv = (ln @ kv_b_w.T).reshape(B, Sq, H, D_NOPE + D_V).transpose(0, 2, 1, 3)
    k_nope, v = kv[..., :D_NOPE], kv[..., D_NOPE:]
    cos = cos_c[pos][:, None]
    sin = sin_c[pos][:, None]

    def rope(x):
        xi = np.concatenate([x[..., 0::2], x[..., 1::2]], -1)
        half = xi.shape[-1] // 2
        rot = np.concatenate([-xi[..., half:], xi[..., :half]], -1)
        return xi * cos + rot * sin

    q_pe, k_pe = rope(q_pe), rope(k_pe)
    query = np.concatenate([q_nope, q_pe], -1)
    key = np.concatenate([k_nope, np.broadcast_to(k_pe, (B, H, Sq, D_ROPE))], -1)
    sc = np.einsum("bhqd,bhkd->bhqk", query, key) * SCALE + mask
    sc = sc - sc.max(-1, keepdims=True)
    a = np.exp(sc)
    a /= a.sum(-1, keepdims=True)
    o = np.einsum("bhqk,bhkd->bhqd", a, v)
    o = o.transpose(0, 2, 1, 3).reshape(B, Sq, H * D_V)
    return (o @ o_w.T).astype(np.float32)


def kernel(**inputs):
    if not _mask_is_causal(inputs["attention_mask"]):
        return _numpy_fallback(inputs)
    pos = np.asarray(inputs["position_ids"])[0].astype(np.int64)
    if pos.shape[0] != S or np.asarray(inputs["hidden_states"]).shape != (1, S, HIDDEN):
        return _numpy_fallback(inputs)
    if np.any(np.asarray(inputs["ln_b"], np.float32) != 0.0):
        return _numpy_fallback(inputs)

    from concourse.bass_utils import run_bass_kernel_spmd

    in_maps = _host_prep(inputs)
    if "prog" not in _CACHE:
        _CACHE["prog"] = _build_program()
    nc = _CACHE["prog"]
    res = run_bass_kernel_spmd(nc, in_maps, core_ids=list(range(NCORES)))
    parts = [np.asarray(res.results[i]["out"], np.float32) for i in range(NCORES)]
    total = np.sum(np.stack(parts, 0), 0, dtype=np.float32)
    return total.reshape(1, S, HIDDEN)
